# revision 4
# baseline (speedup 1.0000x reference)
"""Trainium2 Bass kernel for nn_MultiHeadAttention_81664508166458.

Reference computes a "cross-head" MHA: per (batch, position) the attention
matrix is HxH (H=16 heads), contracting head_dim D=128. Every position is
independent, so we shard the 8192 (batch, position) pairs across 8 cores
(1024 each), fully data-parallel, no collectives.

Host-side preprocessing (part of sharding, not timed device work):
  - weights transposed to [e_in, e_out] (k-major) and cast to bf16
  - RoPE pair permutation baked into Wq/Wk rows: head-local dim d' with
    x0 (even d) in d'=[0,64) and x1 (odd d) in d'=[64,128) so the rotation
    becomes same-partition table multiplies plus a half-swap
  - 1/sqrt(D) attention scale baked into Wq/bq
  - x transposed to [e_in, n] bf16
  - cos/sin tables and the block-diagonal softmax mask precomputed

Device pipeline per core (all matmuls bf16 with fp32 PSUM accumulation):
  1. qT/kT/vT [d, h, n] = W*T.T @ xT   (16 e-tiles x 16 k-tiles, N=512)
  2. RoPE on q,k during PSUM eviction (DVE table mults + ACT half-swap)
  3. per 8-position quad: PE computes the 128x128 "all pairs" (g,h)x(g',t)
     dot products; block-diag mask + exp (+row-sum accum) + normalize;
     PE-transpose att and the v-slice; second matmul gives O^T[d,(g,h)];
     DVE scatters into the layout-scrambled rhs for the final projection
  4. outT[r, (h,t)] = WoT.T @ scr (+bo), DMA to DRAM [E, n] (host transposes)
"""

import numpy as np
import ml_dtypes

B, S, E = 4, 2048, 2048
H, D = 16, 128
NCORES = 8
CORES_PER_BATCH = NCORES // B          # 2
NPOS = S // CORES_PER_BATCH            # 1024 positions per core
THETA = 10000.0
MASK_NEG = -30000.0

BF16 = ml_dtypes.bfloat16

# ---------------------------------------------------------------------------
# Host-side preprocessing
# ---------------------------------------------------------------------------


def _rope_perm():
    """P_IDX[new] = old row index: x0 (even d) -> d'=[0,64), x1 (odd) -> [64,128)."""
    p = np.empty(E, np.int64)
    for h in range(H):
        base = h * D
        i = np.arange(D // 2)
        p[base + i] = base + 2 * i
        p[base + 64 + i] = base + 2 * i + 1
    return p


def _rope_tables(npos, offset):
    """cos table C[p, n] and signed sin table S[p, n], p in [0,128)."""
    inv = 1.0 / (THETA ** (np.arange(0, D, 2, dtype=np.float64) / D))  # [64]
    pos = np.arange(offset, offset + npos, dtype=np.float64)
    fr = np.outer(inv, pos)  # [64, npos]
    c = np.cos(fr).astype(np.float32)
    s = np.sin(fr).astype(np.float32)
    cos_b = np.concatenate([c, c], axis=0)            # [128, npos]
    sin_b = np.concatenate([-s, s], axis=0)           # signed
    return np.ascontiguousarray(cos_b), np.ascontiguousarray(sin_b)


def _blockdiag_mask():
    m = np.full((128, 128), MASK_NEG, np.float32)
    for g in range(8):
        m[g * 16:(g + 1) * 16, g * 16:(g + 1) * 16] = 0.0
    return m


# exact bf16-representable mask magnitude (softmax is shift-invariant, but we
# keep the on-diagonal shift exactly zero: +MASKVAL via matmul, -MASKVAL bias)
MASKVAL = float(np.float32(BF16(30000.0)))


def _mask_mm():
    """K=8 rank-8 matmul operands adding +MASKVAL on the block diagonal.
    maskl[g, p] = MASKVAL if p//16==g else 0 ; maskr[g, f] = 1 if f//16==g."""
    ind = np.zeros((8, 128), np.float32)
    for g in range(8):
        ind[g, g * 16:(g + 1) * 16] = 1.0
    return (ind * MASKVAL).astype(BF16), ind.astype(BF16)


def prepare_host(x, Wq, bq, Wk, bk, Wv, bv, Wo, bo, npos=NPOS, ncores=NCORES):
    """Returns (shared weight arrays dict, list of per-core in_maps)."""
    x = np.asarray(x, np.float32)
    perm = _rope_perm()
    scale = np.float32(1.0 / np.sqrt(D))

    wqt = np.ascontiguousarray((np.asarray(Wq, np.float32)[perm, :] * scale).T).astype(BF16)
    wkt = np.ascontiguousarray(np.asarray(Wk, np.float32)[perm, :].T).astype(BF16)
    wvt = np.ascontiguousarray(np.asarray(Wv, np.float32).T).astype(BF16)
    wot = np.ascontiguousarray(np.asarray(Wo, np.float32).T).astype(BF16)
    bq_p = (np.asarray(bq, np.float32)[perm] * scale).copy()
    bk_p = np.asarray(bk, np.float32)[perm].copy()
    bv_p = np.asarray(bv, np.float32).copy()
    bo_p = np.asarray(bo, np.float32).copy()
    mask = _blockdiag_mask()
    maskl, maskr = _mask_mm()

    in_maps = []
    meta = []
    for c in range(ncores):
        bc = c // CORES_PER_BATCH
        o = (c % CORES_PER_BATCH) * npos
        xc = x[bc, o:o + npos, :]                      # [npos, E]
        xt = np.ascontiguousarray(xc.T).astype(BF16)   # [E, npos]
        cos_b, sin_b = _rope_tables(npos, o)
        in_maps.append({
            "xt": xt, "wqt": wqt, "wkt": wkt, "wvt": wvt, "wot": wot,
            "bq": bq_p, "bk": bk_p, "bv": bv_p, "bo": bo_p,
            "cosb": cos_b, "sinb": sin_b, "mask": mask,
            "maskl": maskl, "maskr": maskr,
            "maskr4": np.ascontiguousarray(np.tile(maskr, (1, 4))),
        })
        meta.append((bc, o))
    return in_maps, meta


def assemble_output(outs, meta, npos=NPOS, layout="h_t"):
    """outs: list of per-core {'outt': [E, npos] f32}. Returns [B, S, E].

    layout "h_t": outt col = h*tw + tc (tc local).
    layout "t_h": outt col = tc*16 + h (scrsplit build).
    """
    full = np.empty((B, S, E), np.float32)
    tw = npos // 16
    for (bc, o), res in zip(meta, outs):
        outt = res["outt"]
        if layout == "h_t":
            v = outt.reshape(E, H, tw)           # [E, h, tc]
            v = np.transpose(v, (1, 2, 0))       # [h, tc, E]
        else:
            v = outt.reshape(E, tw, H)           # [E, tc, h]
            v = np.transpose(v, (2, 1, 0))       # [h, tc, E]
        t0 = o // 16
        for h in range(H):
            full[bc, h * 128 + t0: h * 128 + t0 + tw, :] = v[h]
    return full


# ---------------------------------------------------------------------------
# Numpy emulator of the exact device dataflow (index-math validation)
# ---------------------------------------------------------------------------


def emulate_core(im, npos=NPOS, layout="h_t"):
    f32 = np.float32
    xt = im["xt"].astype(f32)
    qT = (im["wqt"].astype(f32).T @ xt) + im["bq"][:, None]   # [E, n]
    kT = (im["wkt"].astype(f32).T @ xt) + im["bk"][:, None]
    vT = (im["wvt"].astype(f32).T @ xt) + im["bv"][:, None]
    C, Sg = im["cosb"].astype(f32), im["sinb"].astype(f32)

    def rope(t):
        t3 = t.reshape(H, D, npos)                            # [h, d', n]
        sw = np.concatenate([t3[:, 64:, :], t3[:, :64, :]], axis=1)
        r = t3 * C[None] + sw * Sg[None]
        return r.astype(BF16).astype(f32)

    qr, kr = rope(qT), rope(kT)
    vb = vT.astype(BF16).astype(f32).reshape(H, D, npos)
    scr = np.zeros((D, 16, npos), f32)                        # [d, j, h*tw+tc]
    tw = npos // 16
    for g0 in range(npos // 8):
        n0 = 8 * g0
        j0, tc = n0 % 16, g0 // 2
        q_blk = qr[:, :, n0:n0 + 8]                           # [h, d, g]
        k_blk = kr[:, :, n0:n0 + 8]
        lhs = np.transpose(q_blk, (1, 2, 0)).reshape(D, 128)  # [d, (g,h)]
        rhs = np.transpose(k_blk, (1, 2, 0)).reshape(D, 128)  # [d, (g,t)]
        qk = lhs.T @ rhs + im["mask"]
        e = np.exp(qk)
        att = (e / e.sum(1, keepdims=True)).astype(BF16).astype(f32)
        vm = np.transpose(vb[:, :, n0:n0 + 8], (1, 2, 0)).reshape(D, 128)  # [d,(g,t)]
        # out2T[d, (g,h)] = sum_{(g,t)} vm[d, (g,t)] * att[(g,h), (g,t)]
        o2 = vm @ att.T                 # [d, (g,h)]
        o2v = o2.reshape(D, 8, 16)
        if layout == "h_t":
            scr.reshape(D, 16, H, tw)[:, j0:j0 + 8, :, tc] = o2v
        else:
            scr.reshape(D, 16, tw, H)[:, j0:j0 + 8, tc, :] = o2v
    # scr[d, j, col] -> rhs row e=(j*128+d)
    rhs_full = np.transpose(scr, (1, 0, 2)).reshape(16 * D, npos).astype(BF16).astype(f32)
    outt = im["wot"].astype(f32).T @ rhs_full + im["bo"][:, None]
    return {"outt": outt.astype(f32)}


def emulate_full(inputs, npos=NPOS, ncores=NCORES, layout="h_t"):
    in_maps, meta = prepare_host(**inputs, npos=npos, ncores=ncores)
    outs = [emulate_core(im, npos, layout) for im in in_maps]
    return assemble_output(outs, meta, npos, layout)


# ---------------------------------------------------------------------------
# Bass kernel
# ---------------------------------------------------------------------------

_NC_CACHE = {}


def build_nc(npos=NPOS, reps=1, opts=frozenset()):
    import concourse.bass as bass
    import concourse.tile as tile
    from concourse import bacc, mybir
    from concourse.masks import make_identity

    opts = frozenset(opts)
    key = (npos, reps, opts)
    if key in _NC_CACHE:
        return _NC_CACHE[key]

    f32, bf16 = mybir.dt.float32, mybir.dt.bfloat16
    CH = min(512, npos)          # free-dim chunk (one PSUM bank fp32)
    NCH = npos // CH
    TW = npos // 16
    NQ = npos // 8               # number of 8-position quads

    nc = bacc.Bacc("TRN2", target_bir_lowering=False, debug=False)

    xt_d = nc.dram_tensor("xt", [E, npos], bf16, kind="ExternalInput")
    w_d = {
        "q": nc.dram_tensor("wqt", [E, E], bf16, kind="ExternalInput"),
        "k": nc.dram_tensor("wkt", [E, E], bf16, kind="ExternalInput"),
        "v": nc.dram_tensor("wvt", [E, E], bf16, kind="ExternalInput"),
        "o": nc.dram_tensor("wot", [E, E], bf16, kind="ExternalInput"),
    }
    b_d = {
        "q": nc.dram_tensor("bq", [E], f32, kind="ExternalInput"),
        "k": nc.dram_tensor("bk", [E], f32, kind="ExternalInput"),
        "v": nc.dram_tensor("bv", [E], f32, kind="ExternalInput"),
        "o": nc.dram_tensor("bo", [E], f32, kind="ExternalInput"),
    }
    cos_d = nc.dram_tensor("cosb", [128, npos], f32, kind="ExternalInput")
    sin_d = nc.dram_tensor("sinb", [128, npos], f32, kind="ExternalInput")
    mask_d = nc.dram_tensor("mask", [128, 128], f32, kind="ExternalInput")
    maskl_d = nc.dram_tensor("maskl", [8, 128], bf16, kind="ExternalInput")
    maskr_d = nc.dram_tensor("maskr", [8, 128], bf16, kind="ExternalInput")
    maskr4_d = nc.dram_tensor("maskr4", [8, 512], bf16, kind="ExternalInput")
    out_d = nc.dram_tensor("outt", [E, npos], f32, kind="ExternalOutput")

    Exp = mybir.ActivationFunctionType.Exp
    Ident = mybir.ActivationFunctionType.Identity

    def body(tc):
        with (
            tc.tile_pool(name="consts", bufs=1) as consts,
            tc.tile_pool(name="wpool", bufs=2) as wpool,
            tc.tile_pool(name="scrp", bufs=1) as scrp,
            tc.tile_pool(
                name="tmp", bufs=3 if "tmpb3" in opts else 2) as tmp,
            tc.tile_pool(
                name="attp", bufs=4 if "attb4" in opts else 3) as attp,
            tc.tile_pool(
                name="outp", bufs=3 if "outb3" in opts else 2) as outp,
        ):
            cos_sb = consts.tile([128, npos], f32)
            nc.sync.dma_start(cos_sb, cos_d.ap())
            sin_sb = consts.tile([128, npos], f32)
            nc.sync.dma_start(sin_sb, sin_d.ap())
            if "fastmask" in opts:
                ml_sb = consts.tile([8, 128], bf16)
                nc.sync.dma_start(ml_sb, maskl_d.ap())
                mr_sb = consts.tile([8, 128], bf16)
                nc.sync.dma_start(mr_sb, maskr_d.ap())
                ebias = consts.tile([128, 1], f32)
                nc.vector.memset(ebias, -MASKVAL)
                if "maskw" in opts:
                    mr4_sb = consts.tile([8, 512], bf16)
                    nc.sync.dma_start(mr4_sb, maskr4_d.ap())
            else:
                mask_sb = consts.tile([128, 128], f32)
                nc.sync.dma_start(mask_sb, mask_d.ap())
            ident = consts.tile([128, 128], bf16)
            make_identity(nc, ident)
            b_sb = {}
            for p in ("q", "k", "v", "o"):
                b_sb[p] = consts.tile([128, 16], f32, tag=f"b_{p}", name=f"b_{p}")
                nc.sync.dma_start(b_sb[p], b_d[p].ap().rearrange("(t p) -> p t", p=128))

            if "scrsplit" in opts:
                scrA = scrp.tile([128, 16, npos // 2], bf16, tag="scrA")
                scrB = scrp.tile([128, 16, npos // 2], bf16, tag="scrB")
            else:
                scr_sb = scrp.tile([128, 16, npos], bf16)

            with tc.tile_pool(name="qkvp", bufs=1) as qkvp:
                # layout [d, n, h]: per-quad (g,h)/(g,t) views are contiguous
                if "chouter" in opts:
                    # per-chunk tiles so attention can start once a chunk's
                    # projections finish (tile-granular RAW deps)
                    qkv_ch = {
                        p: [
                            qkvp.tile([128, CH, 16], bf16,
                                      tag=f"qkv_{p}{c}", name=f"qkv_{p}{c}")
                            for c in range(NCH)
                        ]
                        for p in ("q", "k", "v")
                    }
                else:
                    qkv_sb = {
                        p: qkvp.tile([128, npos, 16], bf16,
                                     tag=f"qkv_{p}", name=f"qkv_{p}")
                        for p in ("q", "k", "v")
                    }

                # ---------------- phase 1: projections ----------------
                import contextlib
                fuse = "fuse" in opts
                held = []
                xp_ctx = tc.tile_pool(name="xp", bufs=1)
                if fuse:
                    # keep all PSUM pools open across phases (2+4+2=8 banks)
                    # so the scheduler can fill attention-chain PE stalls
                    # with projection/O-proj matmuls
                    ps1_cm = tc.tile_pool(name="ps1", bufs=2, space="PSUM")
                    ps1_h = ps1_cm.__enter__()
                    held.append(ps1_cm)
                    ps1_ctx = contextlib.nullcontext(ps1_h)
                else:
                    ps1_ctx = tc.tile_pool(
                        name="ps1", bufs=4 if "ps1b4" in opts else 3,
                        space="PSUM")
                with (xp_ctx as xp, ps1_ctx as ps1):
                    xt_sb = xp.tile([128, 16, npos], bf16)
                    nc.sync.dma_start(
                        xt_sb, xt_d.ap().rearrange("(kt kp) n -> kp kt n", kp=128))

                    if "chouter" in opts:
                        loop_iter = [
                            (p, ch, mg)
                            for p in ("q", "k", "v")
                            for ch in range(NCH)
                            for mg in range(8)
                        ]
                    else:
                        loop_iter = [
                            (p, None, mg)
                            for p in ("q", "k", "v")
                            for mg in range(8)
                        ]
                    for p, ch_o, mg in loop_iter:
                        wv_d = w_d[p].ap().rearrange("(kt kp) e -> kp kt e", kp=128)
                        if True:
                            w_sb = wpool.tile([128, 16, 256], bf16, tag="w")
                            nc.sync.dma_start(
                                w_sb, wv_d[:, :, mg * 256:(mg + 1) * 256])
                            for mo in range(2):
                                m = mg * 2 + mo
                                for ch in ([ch_o] if ch_o is not None
                                           else range(NCH)):
                                    csl = slice(ch * CH, (ch + 1) * CH)
                                    ps = ps1.tile([128, CH], f32)
                                    for kt in range(16):
                                        nc.tensor.matmul(
                                            ps,
                                            lhsT=w_sb[:, kt, mo * 128:(mo + 1) * 128],
                                            rhs=xt_sb[:, kt, csl],
                                            start=(kt == 0), stop=(kt == 15))
                                    if "chouter" in opts:
                                        qdst = qkv_ch[p][ch][:, :, m]
                                    else:
                                        qdst = qkv_sb[p][:, csl, m]
                                    if p == "v" or "norope" in opts:
                                        nc.scalar.activation(
                                            qdst, ps, Ident,
                                            bias=b_sb["v"][:, m:m + 1])
                                    else:
                                        nc.vector.tensor_scalar_add(
                                            ps, ps, b_sb[p][:, m:m + 1])
                                        t1 = tmp.tile([128, CH], f32, tag="t1")
                                        nc.vector.tensor_mul(t1, ps, cos_sb[:, csl])
                                        tsw = tmp.tile([128, CH], f32, tag="tsw")
                                        nc.scalar.copy(tsw[0:64, :], ps[64:128, :])
                                        nc.scalar.copy(tsw[64:128, :], ps[0:64, :])
                                        nc.vector.tensor_mul(tsw, tsw, sin_sb[:, csl])
                                        nc.vector.tensor_add(qdst, t1, tsw)

                # ---------------- phase 2: attention ----------------
                if "scrsplit" in opts:
                    scr5 = [
                        s.rearrange("p j (t h) -> p j t h", h=16)
                        for s in (scrA, scrB)
                    ]
                else:
                    scr4 = scr_sb.rearrange("p j (h t) -> p j h t", h=16)
                nquads = npos // 8
                vphoist = "vphoist" in opts

                with tc.tile_pool(name="v2p", bufs=1) as v2p:
                    if vphoist:
                        v2_sb = v2p.tile([128, nquads, 128], bf16)
                        with tc.tile_pool(
                                name="vpps", bufs=4, space="PSUM") as vpps:
                            for g0 in range(nquads):
                                n0 = 8 * g0
                                v_v = qkv_sb["v"][:, n0:n0 + 8, :].rearrange(
                                    "d g t -> d (g t)")
                                vp_ps = vpps.tile([128, 128], bf16, tag="vp")
                                nc.tensor.transpose(vp_ps, v_v, ident)
                                nc.vector.tensor_copy(v2_sb[:, g0, :], vp_ps)

                    if fuse:
                        _c = tc.tile_pool(name="ps2", bufs=1, space="PSUM")
                        ps2_h = _c.__enter__()
                        held.append(_c)
                        ps2_cm = contextlib.nullcontext(ps2_h)
                    elif vphoist:
                        ps2_cm = tc.tile_pool(name="ps2", bufs=3, space="PSUM")
                    else:
                        ps2_cm = tc.tile_pool(name="ps2", bufs=2, space="PSUM")
                    with ps2_cm as ps2:
                        if "noatt" in opts:
                            if "scrsplit" in opts:
                                nc.vector.memset(scrA, 0.0)
                                nc.vector.memset(scrB, 0.0)
                            else:
                                nc.vector.memset(scr_sb, 0.0)
                        if "qbatch" in opts:
                            assert {"fastmask", "scrsplit"} <= opts
                            tw2 = TW // 2

                            def qkv_slice(p, n0):
                                if "chouter" in opts:
                                    return qkv_ch[p][n0 // CH][
                                        :, n0 % CH:n0 % CH + 8, :]
                                return qkv_sb[p][:, n0:n0 + 8, :]

                            for a in range(nquads // 4):
                                qk4 = ps2.tile(
                                    [128, 4, 128], f32, tag="qk4",
                                    bufs=3 if "tpsmerge" in opts else None)
                                for qi in range(4):
                                    n0 = 32 * a + 8 * qi
                                    q_v = qkv_slice("q", n0).rearrange(
                                        "d g h -> d (g h)")
                                    k_v = qkv_slice("k", n0).rearrange(
                                        "d g h -> d (g h)")
                                    nc.tensor.matmul(
                                        qk4[:, qi, :], lhsT=q_v, rhs=k_v,
                                        start=(qi == 0), stop=False,
                                        skip_group_check=True)
                                    if "maskw" not in opts:
                                        nc.tensor.matmul(
                                            qk4[:, qi, :], lhsT=ml_sb,
                                            rhs=mr_sb,
                                            start=False, stop=(qi == 3),
                                            skip_group_check=True)
                                if "maskw" in opts:
                                    nc.tensor.matmul(
                                        qk4.rearrange("p q n -> p (q n)"),
                                        lhsT=ml_sb, rhs=mr4_sb,
                                        start=False, stop=True,
                                        skip_group_check=True)
                                att4 = attp.tile([128, 4, 128], bf16, tag="att4")
                                rs4 = attp.tile([128, 4], f32, tag="rs4")
                                if "eacc" in opts:
                                    for qi in range(4):
                                        nc.scalar.activation(
                                            att4[:, qi, :], qk4[:, qi, :],
                                            Exp, bias=ebias,
                                            accum_out=rs4[:, qi:qi + 1])
                                else:
                                    nc.scalar.activation(
                                        att4, qk4, Exp, bias=ebias)
                                    nc.vector.reduce_sum(
                                        out=rs4, in_=att4,
                                        axis=mybir.AxisListType.X)
                                rc4 = attp.tile([128, 4], f32, tag="rc4")
                                nc.vector.reciprocal(rc4, rs4)
                                for qi in range(4):
                                    if "mulact" in opts:
                                        nc.scalar.mul(
                                            att4[:, qi, :], att4[:, qi, :],
                                            rc4[:, qi:qi + 1])
                                    else:
                                        nc.vector.tensor_scalar_mul(
                                            att4[:, qi, :], att4[:, qi, :],
                                            rc4[:, qi:qi + 1])

                                if "tpsmerge" in opts:
                                    tps_ps = ps2.tile(
                                        [128, 8, 128], bf16, tag="tps",
                                        bufs=2)
                                    for qi in range(4):
                                        n0 = 32 * a + 8 * qi
                                        v_v = qkv_sb["v"][
                                            :, n0:n0 + 8, :].rearrange(
                                            "d g t -> d (g t)")
                                        nc.tensor.matmul(
                                            tps_ps[:, 4 + qi, :], lhsT=v_v,
                                            rhs=ident, is_transpose=True,
                                            start=(qi == 0), stop=False,
                                            skip_group_check=True)
                                    for qi in range(4):
                                        nc.tensor.matmul(
                                            tps_ps[:, qi, :],
                                            lhsT=att4[:, qi, :], rhs=ident,
                                            is_transpose=True,
                                            start=False, stop=(qi == 3),
                                            skip_group_check=True)
                                    tps = attp.tile(
                                        [128, 8, 128], bf16, tag="tpss")
                                    if "attcopyact" in opts:
                                        nc.scalar.copy(tps, tps_ps)
                                    else:
                                        nc.vector.tensor_copy(tps, tps_ps)
                                    attT4 = tps[:, 0:4, :]
                                    vp4 = tps[:, 4:8, :]
                                else:
                                    attT4_ps = ps2.tile(
                                        [128, 4, 128], bf16, tag="attT4")
                                    vp4_ps = ps2.tile(
                                        [128, 4, 128], bf16, tag="vp4")
                                    for qi in range(4):
                                        n0 = 32 * a + 8 * qi
                                        nc.tensor.matmul(
                                            attT4_ps[:, qi, :],
                                            lhsT=att4[:, qi, :], rhs=ident,
                                            is_transpose=True,
                                            start=(qi == 0), stop=(qi == 3),
                                            skip_group_check=True)
                                        v_v = qkv_slice("v", n0).rearrange(
                                            "d g t -> d (g t)")
                                        nc.tensor.matmul(
                                            vp4_ps[:, qi, :], lhsT=v_v,
                                            rhs=ident, is_transpose=True,
                                            start=(qi == 0), stop=(qi == 3),
                                            skip_group_check=True)
                                    attT4 = attp.tile(
                                        [128, 4, 128], bf16, tag="attT4s")
                                    if "attcopyact" in opts:
                                        nc.scalar.copy(attT4, attT4_ps)
                                    else:
                                        nc.vector.tensor_copy(attT4, attT4_ps)
                                    vp4 = attp.tile(
                                        [128, 4, 128], bf16, tag="vp4s")
                                    if "vpcopyact" in opts:
                                        nc.scalar.copy(vp4, vp4_ps)
                                    else:
                                        nc.vector.tensor_copy(vp4, vp4_ps)

                                o4_ps = ps2.tile(
                                    [128, 4, 128], f32, tag="o4",
                                    bufs=3 if "tpsmerge" in opts else None)
                                for qi in range(4):
                                    nc.tensor.matmul(
                                        o4_ps[:, qi, :],
                                        lhsT=vp4[:, qi, :],
                                        rhs=attT4[:, qi, :],
                                        start=(qi == 0), stop=(qi == 3),
                                        skip_group_check=True)
                                tc0 = 2 * a
                                half, tcl0 = tc0 // tw2, tc0 % tw2
                                dst = scr5[half][:, :, tcl0:tcl0 + 2, :].rearrange(
                                    "p (jb g) t h -> p t jb g h", jb=2)
                                nc.vector.tensor_copy(
                                    dst,
                                    o4_ps.rearrange(
                                        "p (tb jb) (g h) -> p tb jb g h",
                                        jb=2, h=16))
                            nquads_left = 0
                        else:
                            nquads_left = nquads
                        for g0 in range(
                                0 if "noatt" not in opts and nquads_left else 10**9,
                                nquads_left):
                            n0 = 8 * g0
                            j0, tc_ = n0 % 16, g0 // 2
                            q_v = qkv_sb["q"][:, n0:n0 + 8, :].rearrange(
                                "d g h -> d (g h)")
                            k_v = qkv_sb["k"][:, n0:n0 + 8, :].rearrange(
                                "d g h -> d (g h)")
                            qk_ps = ps2.tile([128, 128], f32, tag="qk")
                            att = attp.tile([128, 128], bf16, tag="att")
                            rs = attp.tile([128, 1], f32, tag="rs")
                            if "fastmask" in opts:
                                nc.tensor.matmul(
                                    qk_ps, lhsT=q_v, rhs=k_v,
                                    start=True, stop=False)
                                nc.tensor.matmul(
                                    qk_ps, lhsT=ml_sb, rhs=mr_sb,
                                    start=False, stop=True)
                                nc.scalar.activation(
                                    att, qk_ps, Exp, bias=ebias, accum_out=rs)
                            else:
                                nc.tensor.matmul(
                                    qk_ps, lhsT=q_v, rhs=k_v,
                                    start=True, stop=True)
                                if "noatt_dve" not in opts:
                                    nc.vector.tensor_add(qk_ps, qk_ps, mask_sb)
                                nc.scalar.activation(att, qk_ps, Exp, accum_out=rs)
                            if "noatt_dve" not in opts:
                                if "divnorm" in opts:
                                    nc.vector.tensor_scalar(
                                        att, att, rs, None,
                                        op0=mybir.AluOpType.divide)
                                else:
                                    rc = attp.tile([128, 1], f32, tag="rc")
                                    nc.vector.reciprocal(rc, rs)
                                    nc.vector.tensor_scalar_mul(att, att, rc)

                            attT_ps = ps2.tile([128, 128], bf16, tag="attT")
                            nc.tensor.transpose(attT_ps, att, ident)
                            attT = attp.tile([128, 128], bf16, tag="attTs")
                            nc.vector.tensor_copy(attT, attT_ps)

                            if vphoist:
                                vp = v2_sb[:, g0, :]
                            else:
                                v_v = qkv_sb["v"][:, n0:n0 + 8, :].rearrange(
                                    "d g t -> d (g t)")
                                vp_ps = ps2.tile([128, 128], bf16, tag="vp")
                                nc.tensor.transpose(vp_ps, v_v, ident)
                                vp = attp.tile([128, 128], bf16, tag="vps")
                                nc.vector.tensor_copy(vp, vp_ps)

                            o_ps = ps2.tile(
                                [128, 128], f32, tag="o",
                                bufs=2 if vphoist else None)
                            nc.tensor.matmul(
                                o_ps, lhsT=vp, rhs=attT, start=True, stop=True)
                            if "scrsplit" in opts:
                                tw2 = TW // 2
                                half, tcl = tc_ // tw2, tc_ % tw2
                                dst = scr5[half][:, j0:j0 + 8, tcl:tcl + 1, :]
                            else:
                                dst = scr4[:, j0:j0 + 8, :, tc_:tc_ + 1]
                            nc.vector.tensor_copy(
                                dst, o_ps.rearrange("p (g h) -> p g h", g=8))

            # ---------------- phase 3: output projection ----------------
            wo_v = w_d["o"].ap().rearrange("(jt jp) r -> jp jt r", jp=128)
            out_v = out_d.ap().rearrange("(rt rp) n -> rp rt n", rp=128)
            if "scrsplit" in opts:
                ch3 = npos // 2
                chunks = [(scrA, slice(0, ch3)), (scrB, slice(ch3, npos))]
            else:
                ch3 = CH
                chunks = [
                    (scr_sb, slice(c * CH, (c + 1) * CH)) for c in range(NCH)]
            if fuse:
                _c3 = tc.tile_pool(name="ps3", bufs=2, space="PSUM")
                ps3_h = _c3.__enter__()
                held.append(_c3)
                ps3_ctx = contextlib.nullcontext(ps3_h)
            else:
                ps3_ctx = tc.tile_pool(name="ps3", bufs=3, space="PSUM")
            with ps3_ctx as ps3:
                for rg in range(8):
                    w_sb = wpool.tile([128, 16, 256], bf16, tag="w")
                    nc.sync.dma_start(w_sb, wo_v[:, :, rg * 256:(rg + 1) * 256])
                    for ro in range(2):
                        r = rg * 2 + ro
                        for src, csl in chunks:
                            ps = ps3.tile([128, ch3], f32)
                            for j in range(16):
                                rhs = (src[:, j, :] if "scrsplit" in opts
                                       else src[:, j, csl])
                                nc.tensor.matmul(
                                    ps,
                                    lhsT=w_sb[:, j, ro * 128:(ro + 1) * 128],
                                    rhs=rhs,
                                    start=(j == 0), stop=(j == 15))
                            o_sb = outp.tile([128, ch3], f32)
                            nc.scalar.activation(
                                o_sb, ps, Ident, bias=b_sb["o"][:, r:r + 1])
                            nc.sync.dma_start(out_v[:, r, csl], o_sb)

            for cm in reversed(held):
                cm.__exit__(None, None, None)

    def body_v2(tc):
        """Dependency-restructured pipeline:

        q-proj, k-proj (m-outer, weights streamed once) -> attention part1
        (qk+mask matmul, exp, normalize -> att_all in SBUF; needs only q,k)
        emitted BEFORE v-proj so part1's ACT/DVE work overlaps v's dense PE
        matmuls -> part2 (transposes + att@v + scatter) -> o-projection in
        two passes (scrA, scrB; Wo streamed twice) so the list scheduler
        fills part2 latency bubbles with o-proj matmuls.
        """
        with (
            tc.tile_pool(name="consts", bufs=1) as consts,
            tc.tile_pool(name="wpool", bufs=2) as wpool,
            tc.tile_pool(name="vpool", bufs=1) as vpool,
            tc.tile_pool(name="attall", bufs=1) as attall,
            tc.tile_pool(name="attp", bufs=3) as attp,
            tc.tile_pool(name="tmp", bufs=2) as tmp,
            tc.tile_pool(name="outp", bufs=2) as outp,
        ):
            cos_sb = consts.tile([128, npos], f32)
            nc.sync.dma_start(cos_sb, cos_d.ap())
            sin_sb = consts.tile([128, npos], f32)
            nc.sync.dma_start(sin_sb, sin_d.ap())
            ml_sb = consts.tile([8, 128], bf16)
            nc.sync.dma_start(ml_sb, maskl_d.ap())
            mr4_sb = consts.tile([8, 512], bf16)
            nc.sync.dma_start(mr4_sb, maskr4_d.ap())
            ebias = consts.tile([128, 1], f32)
            nc.vector.memset(ebias, -MASKVAL)
            ident = consts.tile([128, 128], bf16)
            make_identity(nc, ident)
            b_sb = {}
            for p in ("q", "k", "v", "o"):
                b_sb[p] = consts.tile([128, 16], f32, tag=f"b_{p}", name=f"b_{p}")
                nc.sync.dma_start(b_sb[p], b_d[p].ap().rearrange("(t p) -> p t", p=128))

            att_all = attall.tile([128, NQ // 4, 4, 128], bf16)
            v_sb = vpool.tile([128, npos, 16], bf16, tag="qkv_v", name="qkv_v")

            with (
                tc.tile_pool(name="qkp", bufs=1) as qkp,
                tc.tile_pool(name="xp", bufs=1) as xp,
                tc.tile_pool(name="ps1", bufs=3, space="PSUM") as ps1,
                tc.tile_pool(name="psqk", bufs=3, space="PSUM") as psqk,
            ):
                xt_sb = xp.tile([128, 16, npos], bf16)
                xt_v = xt_d.ap().rearrange("(kt kp) n -> kp kt n", kp=128)
                for ch in range(NCH):
                    csl = slice(ch * CH, (ch + 1) * CH)
                    nc.sync.dma_start(xt_sb[:, :, csl], xt_v[:, :, csl])

                qk_sb = {
                    p: qkp.tile([128, npos, 16], bf16,
                                tag=f"qkv_{p}", name=f"qkv_{p}")
                    for p in ("q", "k")
                }

                def proj(p, dst):
                    wv_d = w_d[p].ap().rearrange("(kt kp) e -> kp kt e", kp=128)
                    for mg in range(8):
                        w_sb = wpool.tile([128, 16, 256], bf16, tag="w")
                        nc.sync.dma_start(
                            w_sb, wv_d[:, :, mg * 256:(mg + 1) * 256])
                        for mo in range(2):
                            m = mg * 2 + mo
                            for ch in range(NCH):
                                csl = slice(ch * CH, (ch + 1) * CH)
                                ps = ps1.tile([128, CH], f32, tag="ps1")
                                for kt in range(16):
                                    nc.tensor.matmul(
                                        ps,
                                        lhsT=w_sb[:, kt, mo * 128:(mo + 1) * 128],
                                        rhs=xt_sb[:, kt, csl],
                                        start=(kt == 0), stop=(kt == 15))
                                qdst = dst[:, csl, m]
                                if p == "v":
                                    nc.scalar.activation(
                                        qdst, ps, Ident,
                                        bias=b_sb["v"][:, m:m + 1])
                                else:
                                    nc.vector.tensor_scalar_add(
                                        ps, ps, b_sb[p][:, m:m + 1])
                                    t1 = tmp.tile([128, CH], f32, tag="t1")
                                    nc.vector.tensor_mul(t1, ps, cos_sb[:, csl])
                                    tsw = tmp.tile([128, CH], f32, tag="tsw")
                                    nc.scalar.copy(tsw[0:64, :], ps[64:128, :])
                                    nc.scalar.copy(tsw[64:128, :], ps[0:64, :])
                                    nc.vector.tensor_mul(tsw, tsw, sin_sb[:, csl])
                                    nc.vector.tensor_add(qdst, t1, tsw)

                proj("q", qk_sb["q"])
                proj("k", qk_sb["k"])

                # attention part1: qk + mask -> exp -> normalize -> att_all
                for a in range(NQ // 4):
                    qk4 = psqk.tile([128, 4, 128], f32, tag="qk4")
                    for qi in range(4):
                        n0 = 32 * a + 8 * qi
                        q_v = qk_sb["q"][:, n0:n0 + 8, :].rearrange(
                            "d g h -> d (g h)")
                        k_v = qk_sb["k"][:, n0:n0 + 8, :].rearrange(
                            "d g h -> d (g h)")
                        nc.tensor.matmul(
                            qk4[:, qi, :], lhsT=q_v, rhs=k_v,
                            start=(qi == 0), stop=False,
                            skip_group_check=True)
                    nc.tensor.matmul(
                        qk4.rearrange("p q n -> p (q n)"),
                        lhsT=ml_sb, rhs=mr4_sb,
                        start=False, stop=True, skip_group_check=True)
                    att4 = att_all[:, a, :, :]
                    nc.scalar.activation(att4, qk4, Exp, bias=ebias)
                    rs4 = attp.tile([128, 4], f32, tag="rs4")
                    nc.vector.reduce_sum(
                        out=rs4, in_=att4, axis=mybir.AxisListType.X)
                    rc4 = attp.tile([128, 4], f32, tag="rc4")
                    nc.vector.reciprocal(rc4, rs4)
                    for qi in range(4):
                        nc.vector.tensor_scalar_mul(
                            att4[:, qi, :], att4[:, qi, :], rc4[:, qi:qi + 1])

                proj("v", v_sb)

            # ---- part2 + o-projection, interleaved by the scheduler ----
            with (
                tc.tile_pool(name="scrp", bufs=1) as scrp,
                tc.tile_pool(name="ps2", bufs=2, space="PSUM") as ps2,
                tc.tile_pool(name="ps3", bufs=3, space="PSUM") as ps3,
            ):
                tw2 = TW // 2
                scrA = scrp.tile([128, 16, npos // 2], bf16, tag="scrA")
                scrB = scrp.tile([128, 16, npos // 2], bf16, tag="scrB")
                scr5 = [
                    s.rearrange("p j (t h) -> p j t h", h=16)
                    for s in (scrA, scrB)
                ]

                def part2(a):
                    tps_ps = ps2.tile([128, 8, 128], bf16, tag="tps")
                    for qi in range(4):
                        n0 = 32 * a + 8 * qi
                        v_v = v_sb[:, n0:n0 + 8, :].rearrange(
                            "d g t -> d (g t)")
                        nc.tensor.matmul(
                            tps_ps[:, 4 + qi, :], lhsT=v_v,
                            rhs=ident, is_transpose=True,
                            start=(qi == 0), stop=False,
                            skip_group_check=True)
                    for qi in range(4):
                        nc.tensor.matmul(
                            tps_ps[:, qi, :],
                            lhsT=att_all[:, a, qi, :], rhs=ident,
                            is_transpose=True,
                            start=False, stop=(qi == 3),
                            skip_group_check=True)
                    tps = attp.tile([128, 8, 128], bf16, tag="tpss")
                    nc.scalar.copy(tps, tps_ps)
                    o4_ps = ps2.tile([128, 4, 128], f32, tag="o4")
                    for qi in range(4):
                        nc.tensor.matmul(
                            o4_ps[:, qi, :],
                            lhsT=tps[:, 4 + qi, :],
                            rhs=tps[:, qi, :],
                            start=(qi == 0), stop=(qi == 3),
                            skip_group_check=True)
                    tc0 = 2 * a
                    half, tcl0 = tc0 // tw2, tc0 % tw2
                    dst = scr5[half][:, :, tcl0:tcl0 + 2, :].rearrange(
                        "p (jb g) t h -> p t jb g h", jb=2)
                    nc.vector.tensor_copy(
                        dst,
                        o4_ps.rearrange(
                            "p (tb jb) (g h) -> p tb jb g h",
                            jb=2, h=16))

                wo_v = w_d["o"].ap().rearrange("(jt jp) r -> jp jt r", jp=128)
                out_v = out_d.ap().rearrange("(rt rp) n -> rp rt n", rp=128)

                def oproj_pass(src, osl):
                    for rg in range(8):
                        w_sb = wpool.tile([128, 16, 256], bf16, tag="w")
                        nc.sync.dma_start(
                            w_sb, wo_v[:, :, rg * 256:(rg + 1) * 256])
                        for ro in range(2):
                            r = rg * 2 + ro
                            ps = ps3.tile([128, npos // 2], f32, tag="ps3")
                            for j in range(16):
                                nc.tensor.matmul(
                                    ps,
                                    lhsT=w_sb[:, j, ro * 128:(ro + 1) * 128],
                                    rhs=src[:, j, :],
                                    start=(j == 0), stop=(j == 15))
                            o_sb = outp.tile([128, npos // 2], f32)
                            nc.scalar.activation(
                                o_sb, ps, Ident, bias=b_sb["o"][:, r:r + 1])
                            nc.sync.dma_start(out_v[:, r, osl], o_sb)

                for a in range(NQ // 8):
                    part2(a)
                oproj_pass(scrA, slice(0, npos // 2))
                for a in range(NQ // 8, NQ // 4):
                    part2(a)
                oproj_pass(scrB, slice(npos // 2, npos))

    with tile.TileContext(nc) as tc:
        for _ in range(reps):
            if "v2" in opts:
                body_v2(tc)
            else:
                body(tc)

    nc.compile()
    _NC_CACHE[key] = nc
    return nc


# ---------------------------------------------------------------------------
# Runner (PJRT via axon, cached jitted callable)
# ---------------------------------------------------------------------------

_RUNNER_CACHE = {}


def make_runner(nc, ncores=NCORES):
    """Returns run(in_maps) -> list of per-core output dicts.

    Mirrors bass2jax.run_bass_via_pjrt but caches the jitted callable and
    does NOT donate output buffers (kernel writes every output element), so
    repeated timed calls don't re-trace or re-transfer.
    """
    key = id(nc)
    if key in _RUNNER_CACHE:
        return _RUNNER_CACHE[key]

    import jax
    import numpy as _np
    from jax.sharding import Mesh, PartitionSpec
    from jax.experimental.shard_map import shard_map
    from concourse import mybir
    from concourse import bass2jax
    from concourse.bass2jax import (
        _bass_exec_p, install_neuronx_cc_hook, partition_id_tensor)

    install_neuronx_cc_hook()

    partition_name = (
        nc.partition_id_tensor.name if nc.partition_id_tensor else None)
    in_names, out_names, out_avals, zero_outs = [], [], [], []
    for alloc in nc.m.functions[0].allocations:
        if not isinstance(alloc, mybir.MemoryLocationSet):
            continue
        name = alloc.memorylocations[0].name
        if alloc.kind == "ExternalInput":
            if name == partition_name:
                continue
            in_names.append(name)
        elif alloc.kind == "ExternalOutput":
            shape = tuple(alloc.tensor_shape)
            dtype = mybir.dt.np(alloc.dtype)
            out_names.append(name)
            out_avals.append(jax.core.ShapedArray(shape, dtype))
            zero_outs.append(_np.zeros(shape, dtype))
    n_params = len(in_names)
    all_in_names = in_names + out_names
    if partition_name is not None:
        all_in_names = all_in_names + [partition_name]

    def _body(*args):
        operands = list(args)
        if partition_name is not None:
            operands.append(partition_id_tensor())
        outs = _bass_exec_p.bind(
            *operands,
            out_avals=tuple(out_avals),
            in_names=tuple(all_in_names),
            out_names=tuple(out_names),
            lowering_input_output_aliases=(),
            sim_require_finite=True,
            sim_require_nnan=True,
            nc=nc,
        )
        return tuple(outs)

    devices = jax.devices()[:ncores]
    mesh = Mesh(np.asarray(devices), ("core",))
    n_outs = len(out_names)
    jitted = jax.jit(
        shard_map(
            _body, mesh=mesh,
            in_specs=(PartitionSpec("core"),) * (n_params + n_outs),
            out_specs=(PartitionSpec("core"),) * n_outs,
            check_rep=False,
        ),
        keep_unused=True,
    )

    zeros_dev = [
        jax.device_put(
            _np.zeros((ncores * z.shape[0], *z.shape[1:]), z.dtype))
        for z in zero_outs
    ]

    def put(in_maps):
        concat = [
            _np.concatenate([_np.asarray(m[name]) for m in in_maps], axis=0)
            for name in in_names
        ]
        return [jax.device_put(a) for a in concat]

    def run_dev(in_dev):
        outs = jitted(*in_dev, *zeros_dev)
        jax.block_until_ready(outs)
        return outs

    def run(in_maps):
        outs = run_dev(put(in_maps))
        res = []
        for c in range(len(in_maps)):
            res.append({
                name: _np.asarray(outs[i]).reshape(
                    len(in_maps), *out_avals[i].shape)[c]
                for i, name in enumerate(out_names)
            })
        return res

    run.put = put
    run.run_dev = run_dev
    run.out_names = out_names
    _RUNNER_CACHE[key] = run
    return run


DEFAULT_OPTS = frozenset({"v2"})


def kernel(**inputs) -> np.ndarray:
    in_maps, meta = prepare_host(**{k: np.asarray(v) for k, v in inputs.items()})
    nc = build_nc(NPOS, opts=DEFAULT_OPTS)
    run = make_runner(nc, NCORES)
    outs = run(in_maps)
    layout = ("t_h" if ("scrsplit" in DEFAULT_OPTS or "v2" in DEFAULT_OPTS)
              else "h_t")
    return assemble_output(outs, meta, NPOS, layout)



# revision 27
# speedup vs baseline: 1.0242x; 1.0242x over previous
"""Trainium2 Bass kernel for nn_MultiHeadAttention_81664508166458.

Reference computes a "cross-head" MHA: per (batch, position) the attention
matrix is HxH (H=16 heads), contracting head_dim D=128. Every position is
independent, so we shard the 8192 (batch, position) pairs across 8 cores
(1024 each), fully data-parallel, no collectives.

Host-side preprocessing (part of sharding, not timed device work):
  - weights transposed to [e_in, e_out] (k-major) and cast to bf16
  - RoPE pair permutation baked into Wq/Wk rows: head-local dim d' with
    x0 (even d) in d'=[0,64) and x1 (odd d) in d'=[64,128) so the rotation
    becomes same-partition table multiplies plus a half-swap
  - 1/sqrt(D) attention scale baked into Wq/bq
  - x transposed to [e_in, n] bf16
  - cos/sin tables and the block-diagonal softmax mask precomputed

Device pipeline per core (all matmuls bf16 with fp32 PSUM accumulation):
  1. qT/kT/vT [d, h, n] = W*T.T @ xT   (16 e-tiles x 16 k-tiles, N=512)
  2. RoPE on q,k during PSUM eviction (DVE table mults + ACT half-swap)
  3. per 8-position quad: PE computes the 128x128 "all pairs" (g,h)x(g',t)
     dot products; block-diag mask + exp (+row-sum accum) + normalize;
     PE-transpose att and the v-slice; second matmul gives O^T[d,(g,h)];
     DVE scatters into the layout-scrambled rhs for the final projection
  4. outT[r, (h,t)] = WoT.T @ scr (+bo), DMA to DRAM [E, n] (host transposes)
"""

import numpy as np
import ml_dtypes

B, S, E = 4, 2048, 2048
H, D = 16, 128
NCORES = 8
CORES_PER_BATCH = NCORES // B          # 2
NPOS = S // CORES_PER_BATCH            # 1024 positions per core
THETA = 10000.0
MASK_NEG = -30000.0

BF16 = ml_dtypes.bfloat16

# ---------------------------------------------------------------------------
# Host-side preprocessing
# ---------------------------------------------------------------------------


def _rope_perm():
    """P_IDX[new] = old row index: x0 (even d) -> d'=[0,64), x1 (odd) -> [64,128)."""
    p = np.empty(E, np.int64)
    for h in range(H):
        base = h * D
        i = np.arange(D // 2)
        p[base + i] = base + 2 * i
        p[base + 64 + i] = base + 2 * i + 1
    return p


def _rope_tables(npos, offset):
    """cos table C[p, n] and signed sin table S[p, n], p in [0,128)."""
    inv = 1.0 / (THETA ** (np.arange(0, D, 2, dtype=np.float64) / D))  # [64]
    pos = np.arange(offset, offset + npos, dtype=np.float64)
    fr = np.outer(inv, pos)  # [64, npos]
    c = np.cos(fr).astype(np.float32)
    s = np.sin(fr).astype(np.float32)
    cos_b = np.concatenate([c, c], axis=0)            # [128, npos]
    sin_b = np.concatenate([-s, s], axis=0)           # signed
    return np.ascontiguousarray(cos_b), np.ascontiguousarray(sin_b)


def _blockdiag_mask():
    m = np.full((128, 128), MASK_NEG, np.float32)
    for g in range(8):
        m[g * 16:(g + 1) * 16, g * 16:(g + 1) * 16] = 0.0
    return m


# exact bf16-representable mask magnitude (softmax is shift-invariant, but we
# keep the on-diagonal shift exactly zero: +MASKVAL via matmul, -MASKVAL bias)
MASKVAL = float(np.float32(BF16(30000.0)))


def _mask_mm():
    """K=8 rank-8 matmul operands adding +MASKVAL on the block diagonal.
    maskl[g, p] = MASKVAL if p//16==g else 0 ; maskr[g, f] = 1 if f//16==g."""
    ind = np.zeros((8, 128), np.float32)
    for g in range(8):
        ind[g, g * 16:(g + 1) * 16] = 1.0
    return (ind * MASKVAL).astype(BF16), ind.astype(BF16)


def _repack_w(wt):
    """[E_in=(kt kp), E_out=(mg e)] -> [mg, kp, kt, e] with e=256."""
    w4 = np.asarray(wt).reshape(16, 128, 8, 256)      # kt kp mg e
    return np.ascontiguousarray(np.transpose(w4, (2, 1, 0, 3)))


def prepare_host(x, Wq, bq, Wk, bk, Wv, bv, Wo, bo, npos=NPOS, ncores=NCORES):
    """Returns (shared weight arrays dict, list of per-core in_maps)."""
    x = np.asarray(x, np.float32)
    perm = _rope_perm()
    scale = np.float32(1.0 / np.sqrt(D))

    wqt = np.ascontiguousarray((np.asarray(Wq, np.float32)[perm, :] * scale).T).astype(BF16)
    wkt = np.ascontiguousarray(np.asarray(Wk, np.float32)[perm, :].T).astype(BF16)
    wvt = np.ascontiguousarray(np.asarray(Wv, np.float32).T).astype(BF16)
    wot = np.ascontiguousarray(np.asarray(Wo, np.float32).T).astype(BF16)
    bq_p = (np.asarray(bq, np.float32)[perm] * scale).copy()
    bk_p = np.asarray(bk, np.float32)[perm].copy()
    bv_p = np.asarray(bv, np.float32).copy()
    bo_p = np.asarray(bo, np.float32).copy()
    mask = _blockdiag_mask()
    maskl, maskr = _mask_mm()

    in_maps = []
    meta = []
    for c in range(ncores):
        bc = c // CORES_PER_BATCH
        o = (c % CORES_PER_BATCH) * npos
        xc = x[bc, o:o + npos, :]                      # [npos, E]
        xt = np.ascontiguousarray(xc.T).astype(BF16)   # [E, npos]
        cos_b, sin_b = _rope_tables(npos, o)
        in_maps.append({
            "xt": xt, "wqt": wqt, "wkt": wkt, "wvt": wvt, "wot": wot,
            "bq": bq_p, "bk": bk_p, "bv": bv_p, "bo": bo_p,
            "cosb": cos_b, "sinb": sin_b, "mask": mask,
            "maskl": maskl, "maskr": maskr,
            "maskr4": np.ascontiguousarray(np.tile(maskr, (1, 4))),
            # weights repacked [mg, kp, kt, e256] so each per-partition DMA
            # row is 16*256*2 = 8KB contiguous (full DMA line efficiency)
            "wqtr": _repack_w(wqt), "wktr": _repack_w(wkt),
            "wvtr": _repack_w(wvt), "wotr": _repack_w(wot),
        })
        meta.append((bc, o))
    return in_maps, meta


def assemble_output(outs, meta, npos=NPOS, layout="h_t"):
    """outs: list of per-core {'outt': [E, npos] f32}. Returns [B, S, E].

    layout "h_t": outt col = h*tw + tc (tc local).
    layout "t_h": outt col = tc*16 + h (scrsplit build).
    """
    full = np.empty((B, S, E), np.float32)
    tw = npos // 16
    for (bc, o), res in zip(meta, outs):
        outt = res["outt"]
        if layout == "h_t":
            v = outt.reshape(E, H, tw)           # [E, h, tc]
            v = np.transpose(v, (1, 2, 0))       # [h, tc, E]
        else:
            v = outt.reshape(E, tw, H)           # [E, tc, h]
            v = np.transpose(v, (2, 1, 0))       # [h, tc, E]
        t0 = o // 16
        for h in range(H):
            full[bc, h * 128 + t0: h * 128 + t0 + tw, :] = v[h]
    return full


# ---------------------------------------------------------------------------
# Numpy emulator of the exact device dataflow (index-math validation)
# ---------------------------------------------------------------------------


def emulate_core(im, npos=NPOS, layout="h_t"):
    f32 = np.float32
    xt = im["xt"].astype(f32)
    qT = (im["wqt"].astype(f32).T @ xt) + im["bq"][:, None]   # [E, n]
    kT = (im["wkt"].astype(f32).T @ xt) + im["bk"][:, None]
    vT = (im["wvt"].astype(f32).T @ xt) + im["bv"][:, None]
    C, Sg = im["cosb"].astype(f32), im["sinb"].astype(f32)

    def rope(t):
        t3 = t.reshape(H, D, npos)                            # [h, d', n]
        sw = np.concatenate([t3[:, 64:, :], t3[:, :64, :]], axis=1)
        r = t3 * C[None] + sw * Sg[None]
        return r.astype(BF16).astype(f32)

    qr, kr = rope(qT), rope(kT)
    vb = vT.astype(BF16).astype(f32).reshape(H, D, npos)
    scr = np.zeros((D, 16, npos), f32)                        # [d, j, h*tw+tc]
    tw = npos // 16
    for g0 in range(npos // 8):
        n0 = 8 * g0
        j0, tc = n0 % 16, g0 // 2
        q_blk = qr[:, :, n0:n0 + 8]                           # [h, d, g]
        k_blk = kr[:, :, n0:n0 + 8]
        lhs = np.transpose(q_blk, (1, 2, 0)).reshape(D, 128)  # [d, (g,h)]
        rhs = np.transpose(k_blk, (1, 2, 0)).reshape(D, 128)  # [d, (g,t)]
        qk = lhs.T @ rhs + im["mask"]
        e = np.exp(qk)
        att = (e / e.sum(1, keepdims=True)).astype(BF16).astype(f32)
        vm = np.transpose(vb[:, :, n0:n0 + 8], (1, 2, 0)).reshape(D, 128)  # [d,(g,t)]
        # out2T[d, (g,h)] = sum_{(g,t)} vm[d, (g,t)] * att[(g,h), (g,t)]
        o2 = vm @ att.T                 # [d, (g,h)]
        o2v = o2.reshape(D, 8, 16)
        if layout == "h_t":
            scr.reshape(D, 16, H, tw)[:, j0:j0 + 8, :, tc] = o2v
        else:
            scr.reshape(D, 16, tw, H)[:, j0:j0 + 8, tc, :] = o2v
    # scr[d, j, col] -> rhs row e=(j*128+d)
    rhs_full = np.transpose(scr, (1, 0, 2)).reshape(16 * D, npos).astype(BF16).astype(f32)
    outt = im["wot"].astype(f32).T @ rhs_full + im["bo"][:, None]
    return {"outt": outt.astype(f32)}


def emulate_full(inputs, npos=NPOS, ncores=NCORES, layout="h_t"):
    in_maps, meta = prepare_host(**inputs, npos=npos, ncores=ncores)
    outs = [emulate_core(im, npos, layout) for im in in_maps]
    return assemble_output(outs, meta, npos, layout)


# ---------------------------------------------------------------------------
# Bass kernel
# ---------------------------------------------------------------------------

_NC_CACHE = {}


def build_nc(npos=NPOS, reps=1, opts=frozenset()):
    import concourse.bass as bass
    import concourse.tile as tile
    from concourse import bacc, mybir
    from concourse.masks import make_identity

    opts = frozenset(opts)
    key = (npos, reps, opts)
    if key in _NC_CACHE:
        return _NC_CACHE[key]

    f32, bf16 = mybir.dt.float32, mybir.dt.bfloat16
    CH = 256 if "ch256" in opts else min(512, npos)  # free-dim chunk
    NCH = npos // CH
    TW = npos // 16
    NQ = npos // 8               # number of 8-position quads

    nc = bacc.Bacc("TRN2", target_bir_lowering=False, debug=False)

    xt_d = nc.dram_tensor("xt", [E, npos], bf16, kind="ExternalInput")
    w_d = {
        "q": nc.dram_tensor("wqt", [E, E], bf16, kind="ExternalInput"),
        "k": nc.dram_tensor("wkt", [E, E], bf16, kind="ExternalInput"),
        "v": nc.dram_tensor("wvt", [E, E], bf16, kind="ExternalInput"),
        "o": nc.dram_tensor("wot", [E, E], bf16, kind="ExternalInput"),
    }
    wr_d = {
        "q": nc.dram_tensor("wqtr", [8, 128, 16, 256], bf16, kind="ExternalInput"),
        "k": nc.dram_tensor("wktr", [8, 128, 16, 256], bf16, kind="ExternalInput"),
        "v": nc.dram_tensor("wvtr", [8, 128, 16, 256], bf16, kind="ExternalInput"),
        "o": nc.dram_tensor("wotr", [8, 128, 16, 256], bf16, kind="ExternalInput"),
    }
    b_d = {
        "q": nc.dram_tensor("bq", [E], f32, kind="ExternalInput"),
        "k": nc.dram_tensor("bk", [E], f32, kind="ExternalInput"),
        "v": nc.dram_tensor("bv", [E], f32, kind="ExternalInput"),
        "o": nc.dram_tensor("bo", [E], f32, kind="ExternalInput"),
    }
    cos_d = nc.dram_tensor("cosb", [128, npos], f32, kind="ExternalInput")
    sin_d = nc.dram_tensor("sinb", [128, npos], f32, kind="ExternalInput")
    mask_d = nc.dram_tensor("mask", [128, 128], f32, kind="ExternalInput")
    maskl_d = nc.dram_tensor("maskl", [8, 128], bf16, kind="ExternalInput")
    maskr_d = nc.dram_tensor("maskr", [8, 128], bf16, kind="ExternalInput")
    maskr4_d = nc.dram_tensor("maskr4", [8, 512], bf16, kind="ExternalInput")
    out_dt = bf16 if "obf16" in opts else f32
    out_d = nc.dram_tensor("outt", [E, npos], out_dt, kind="ExternalOutput")

    Exp = mybir.ActivationFunctionType.Exp
    Ident = mybir.ActivationFunctionType.Identity

    def body(tc):
        with (
            tc.tile_pool(name="consts", bufs=1) as consts,
            tc.tile_pool(name="wpool", bufs=2) as wpool,
            tc.tile_pool(name="scrp", bufs=1) as scrp,
            tc.tile_pool(
                name="tmp", bufs=3 if "tmpb3" in opts else 2) as tmp,
            tc.tile_pool(
                name="attp", bufs=4 if "attb4" in opts else 3) as attp,
            tc.tile_pool(
                name="outp", bufs=3 if "outb3" in opts else 2) as outp,
        ):
            cos_sb = consts.tile([128, npos], f32)
            nc.sync.dma_start(cos_sb, cos_d.ap())
            sin_sb = consts.tile([128, npos], f32)
            nc.sync.dma_start(sin_sb, sin_d.ap())
            if "fastmask" in opts:
                ml_sb = consts.tile([8, 128], bf16)
                nc.sync.dma_start(ml_sb, maskl_d.ap())
                mr_sb = consts.tile([8, 128], bf16)
                nc.sync.dma_start(mr_sb, maskr_d.ap())
                ebias = consts.tile([128, 1], f32)
                nc.vector.memset(ebias, -MASKVAL)
                if "maskw" in opts:
                    mr4_sb = consts.tile([8, 512], bf16)
                    nc.sync.dma_start(mr4_sb, maskr4_d.ap())
            else:
                mask_sb = consts.tile([128, 128], f32)
                nc.sync.dma_start(mask_sb, mask_d.ap())
            ident = consts.tile([128, 128], bf16)
            make_identity(nc, ident)
            b_sb = {}
            for p in ("q", "k", "v", "o"):
                b_sb[p] = consts.tile([128, 16], f32, tag=f"b_{p}", name=f"b_{p}")
                nc.sync.dma_start(b_sb[p], b_d[p].ap().rearrange("(t p) -> p t", p=128))

            if "scrsplit" in opts:
                scrA = scrp.tile([128, 16, npos // 2], bf16, tag="scrA")
                scrB = scrp.tile([128, 16, npos // 2], bf16, tag="scrB")
            else:
                scr_sb = scrp.tile([128, 16, npos], bf16)

            with tc.tile_pool(name="qkvp", bufs=1) as qkvp:
                # layout [d, n, h]: per-quad (g,h)/(g,t) views are contiguous
                if "chouter" in opts:
                    # per-chunk tiles so attention can start once a chunk's
                    # projections finish (tile-granular RAW deps)
                    qkv_ch = {
                        p: [
                            qkvp.tile([128, CH, 16], bf16,
                                      tag=f"qkv_{p}{c}", name=f"qkv_{p}{c}")
                            for c in range(NCH)
                        ]
                        for p in ("q", "k", "v")
                    }
                else:
                    qkv_sb = {
                        p: qkvp.tile([128, npos, 16], bf16,
                                     tag=f"qkv_{p}", name=f"qkv_{p}")
                        for p in ("q", "k", "v")
                    }

                # ---------------- phase 1: projections ----------------
                import contextlib
                fuse = "fuse" in opts
                held = []
                xp_ctx = tc.tile_pool(name="xp", bufs=1)
                if fuse:
                    # keep all PSUM pools open across phases (2+4+2=8 banks)
                    # so the scheduler can fill attention-chain PE stalls
                    # with projection/O-proj matmuls
                    ps1_cm = tc.tile_pool(name="ps1", bufs=2, space="PSUM")
                    ps1_h = ps1_cm.__enter__()
                    held.append(ps1_cm)
                    ps1_ctx = contextlib.nullcontext(ps1_h)
                else:
                    ps1_ctx = tc.tile_pool(
                        name="ps1", bufs=4 if "ps1b4" in opts else 3,
                        space="PSUM")
                with (xp_ctx as xp, ps1_ctx as ps1):
                    xt_sb = xp.tile([128, 16, npos], bf16)
                    nc.sync.dma_start(
                        xt_sb, xt_d.ap().rearrange("(kt kp) n -> kp kt n", kp=128))

                    if "chouter" in opts:
                        loop_iter = [
                            (p, ch, mg)
                            for p in ("q", "k", "v")
                            for ch in range(NCH)
                            for mg in range(8)
                        ]
                    else:
                        loop_iter = [
                            (p, None, mg)
                            for p in ("q", "k", "v")
                            for mg in range(8)
                        ]
                    for p, ch_o, mg in loop_iter:
                        wv_d = w_d[p].ap().rearrange("(kt kp) e -> kp kt e", kp=128)
                        if True:
                            w_sb = wpool.tile([128, 16, 256], bf16, tag="w")
                            nc.sync.dma_start(
                                w_sb, wv_d[:, :, mg * 256:(mg + 1) * 256])
                            for mo in range(2):
                                m = mg * 2 + mo
                                for ch in ([ch_o] if ch_o is not None
                                           else range(NCH)):
                                    csl = slice(ch * CH, (ch + 1) * CH)
                                    ps = ps1.tile([128, CH], f32)
                                    for kt in range(16):
                                        nc.tensor.matmul(
                                            ps,
                                            lhsT=w_sb[:, kt, mo * 128:(mo + 1) * 128],
                                            rhs=xt_sb[:, kt, csl],
                                            start=(kt == 0), stop=(kt == 15))
                                    if "chouter" in opts:
                                        qdst = qkv_ch[p][ch][:, :, m]
                                    else:
                                        qdst = qkv_sb[p][:, csl, m]
                                    if p == "v" or "norope" in opts:
                                        nc.scalar.activation(
                                            qdst, ps, Ident,
                                            bias=b_sb["v"][:, m:m + 1])
                                    else:
                                        nc.vector.tensor_scalar_add(
                                            ps, ps, b_sb[p][:, m:m + 1])
                                        t1 = tmp.tile([128, CH], f32, tag="t1")
                                        nc.vector.tensor_mul(t1, ps, cos_sb[:, csl])
                                        tsw = tmp.tile([128, CH], f32, tag="tsw")
                                        nc.scalar.copy(tsw[0:64, :], ps[64:128, :])
                                        nc.scalar.copy(tsw[64:128, :], ps[0:64, :])
                                        nc.vector.tensor_mul(tsw, tsw, sin_sb[:, csl])
                                        nc.vector.tensor_add(qdst, t1, tsw)

                # ---------------- phase 2: attention ----------------
                if "scrsplit" in opts:
                    scr5 = [
                        s.rearrange("p j (t h) -> p j t h", h=16)
                        for s in (scrA, scrB)
                    ]
                else:
                    scr4 = scr_sb.rearrange("p j (h t) -> p j h t", h=16)
                nquads = npos // 8
                vphoist = "vphoist" in opts

                with tc.tile_pool(name="v2p", bufs=1) as v2p:
                    if vphoist:
                        v2_sb = v2p.tile([128, nquads, 128], bf16)
                        with tc.tile_pool(
                                name="vpps", bufs=4, space="PSUM") as vpps:
                            for g0 in range(nquads):
                                n0 = 8 * g0
                                v_v = qkv_sb["v"][:, n0:n0 + 8, :].rearrange(
                                    "d g t -> d (g t)")
                                vp_ps = vpps.tile([128, 128], bf16, tag="vp")
                                nc.tensor.transpose(vp_ps, v_v, ident)
                                nc.vector.tensor_copy(v2_sb[:, g0, :], vp_ps)

                    if fuse:
                        _c = tc.tile_pool(name="ps2", bufs=1, space="PSUM")
                        ps2_h = _c.__enter__()
                        held.append(_c)
                        ps2_cm = contextlib.nullcontext(ps2_h)
                    elif vphoist:
                        ps2_cm = tc.tile_pool(name="ps2", bufs=3, space="PSUM")
                    else:
                        ps2_cm = tc.tile_pool(name="ps2", bufs=2, space="PSUM")
                    with ps2_cm as ps2:
                        if "noatt" in opts:
                            if "scrsplit" in opts:
                                nc.vector.memset(scrA, 0.0)
                                nc.vector.memset(scrB, 0.0)
                            else:
                                nc.vector.memset(scr_sb, 0.0)
                        if "qbatch" in opts:
                            assert {"fastmask", "scrsplit"} <= opts
                            tw2 = TW // 2

                            def qkv_slice(p, n0):
                                if "chouter" in opts:
                                    return qkv_ch[p][n0 // CH][
                                        :, n0 % CH:n0 % CH + 8, :]
                                return qkv_sb[p][:, n0:n0 + 8, :]

                            for a in range(nquads // 4):
                                qk4 = ps2.tile(
                                    [128, 4, 128], f32, tag="qk4",
                                    bufs=3 if "tpsmerge" in opts else None)
                                for qi in range(4):
                                    n0 = 32 * a + 8 * qi
                                    q_v = qkv_slice("q", n0).rearrange(
                                        "d g h -> d (g h)")
                                    k_v = qkv_slice("k", n0).rearrange(
                                        "d g h -> d (g h)")
                                    nc.tensor.matmul(
                                        qk4[:, qi, :], lhsT=q_v, rhs=k_v,
                                        start=(qi == 0), stop=False,
                                        skip_group_check=True)
                                    if "maskw" not in opts:
                                        nc.tensor.matmul(
                                            qk4[:, qi, :], lhsT=ml_sb,
                                            rhs=mr_sb,
                                            start=False, stop=(qi == 3),
                                            skip_group_check=True)
                                if "maskw" in opts:
                                    nc.tensor.matmul(
                                        qk4.rearrange("p q n -> p (q n)"),
                                        lhsT=ml_sb, rhs=mr4_sb,
                                        start=False, stop=True,
                                        skip_group_check=True)
                                att4 = attp.tile([128, 4, 128], bf16, tag="att4")
                                rs4 = attp.tile([128, 4], f32, tag="rs4")
                                if "eacc" in opts:
                                    for qi in range(4):
                                        nc.scalar.activation(
                                            att4[:, qi, :], qk4[:, qi, :],
                                            Exp, bias=ebias,
                                            accum_out=rs4[:, qi:qi + 1])
                                else:
                                    nc.scalar.activation(
                                        att4, qk4, Exp, bias=ebias)
                                    nc.vector.reduce_sum(
                                        out=rs4, in_=att4,
                                        axis=mybir.AxisListType.X)
                                rc4 = attp.tile([128, 4], f32, tag="rc4")
                                nc.vector.reciprocal(rc4, rs4)
                                for qi in range(4):
                                    if "mulact" in opts:
                                        nc.scalar.mul(
                                            att4[:, qi, :], att4[:, qi, :],
                                            rc4[:, qi:qi + 1])
                                    else:
                                        nc.vector.tensor_scalar_mul(
                                            att4[:, qi, :], att4[:, qi, :],
                                            rc4[:, qi:qi + 1])

                                if "tpsmerge" in opts:
                                    tps_ps = ps2.tile(
                                        [128, 8, 128], bf16, tag="tps",
                                        bufs=2)
                                    for qi in range(4):
                                        n0 = 32 * a + 8 * qi
                                        v_v = qkv_sb["v"][
                                            :, n0:n0 + 8, :].rearrange(
                                            "d g t -> d (g t)")
                                        nc.tensor.matmul(
                                            tps_ps[:, 4 + qi, :], lhsT=v_v,
                                            rhs=ident, is_transpose=True,
                                            start=(qi == 0), stop=False,
                                            skip_group_check=True)
                                    for qi in range(4):
                                        nc.tensor.matmul(
                                            tps_ps[:, qi, :],
                                            lhsT=att4[:, qi, :], rhs=ident,
                                            is_transpose=True,
                                            start=False, stop=(qi == 3),
                                            skip_group_check=True)
                                    tps = attp.tile(
                                        [128, 8, 128], bf16, tag="tpss")
                                    if "attcopyact" in opts:
                                        nc.scalar.copy(tps, tps_ps)
                                    else:
                                        nc.vector.tensor_copy(tps, tps_ps)
                                    attT4 = tps[:, 0:4, :]
                                    vp4 = tps[:, 4:8, :]
                                else:
                                    attT4_ps = ps2.tile(
                                        [128, 4, 128], bf16, tag="attT4")
                                    vp4_ps = ps2.tile(
                                        [128, 4, 128], bf16, tag="vp4")
                                    for qi in range(4):
                                        n0 = 32 * a + 8 * qi
                                        nc.tensor.matmul(
                                            attT4_ps[:, qi, :],
                                            lhsT=att4[:, qi, :], rhs=ident,
                                            is_transpose=True,
                                            start=(qi == 0), stop=(qi == 3),
                                            skip_group_check=True)
                                        v_v = qkv_slice("v", n0).rearrange(
                                            "d g t -> d (g t)")
                                        nc.tensor.matmul(
                                            vp4_ps[:, qi, :], lhsT=v_v,
                                            rhs=ident, is_transpose=True,
                                            start=(qi == 0), stop=(qi == 3),
                                            skip_group_check=True)
                                    attT4 = attp.tile(
                                        [128, 4, 128], bf16, tag="attT4s")
                                    if "attcopyact" in opts:
                                        nc.scalar.copy(attT4, attT4_ps)
                                    else:
                                        nc.vector.tensor_copy(attT4, attT4_ps)
                                    vp4 = attp.tile(
                                        [128, 4, 128], bf16, tag="vp4s")
                                    if "vpcopyact" in opts:
                                        nc.scalar.copy(vp4, vp4_ps)
                                    else:
                                        nc.vector.tensor_copy(vp4, vp4_ps)

                                o4_ps = ps2.tile(
                                    [128, 4, 128], f32, tag="o4",
                                    bufs=3 if "tpsmerge" in opts else None)
                                for qi in range(4):
                                    nc.tensor.matmul(
                                        o4_ps[:, qi, :],
                                        lhsT=vp4[:, qi, :],
                                        rhs=attT4[:, qi, :],
                                        start=(qi == 0), stop=(qi == 3),
                                        skip_group_check=True)
                                tc0 = 2 * a
                                half, tcl0 = tc0 // tw2, tc0 % tw2
                                dst = scr5[half][:, :, tcl0:tcl0 + 2, :].rearrange(
                                    "p (jb g) t h -> p t jb g h", jb=2)
                                nc.vector.tensor_copy(
                                    dst,
                                    o4_ps.rearrange(
                                        "p (tb jb) (g h) -> p tb jb g h",
                                        jb=2, h=16))
                            nquads_left = 0
                        else:
                            nquads_left = nquads
                        for g0 in range(
                                0 if "noatt" not in opts and nquads_left else 10**9,
                                nquads_left):
                            n0 = 8 * g0
                            j0, tc_ = n0 % 16, g0 // 2
                            q_v = qkv_sb["q"][:, n0:n0 + 8, :].rearrange(
                                "d g h -> d (g h)")
                            k_v = qkv_sb["k"][:, n0:n0 + 8, :].rearrange(
                                "d g h -> d (g h)")
                            qk_ps = ps2.tile([128, 128], f32, tag="qk")
                            att = attp.tile([128, 128], bf16, tag="att")
                            rs = attp.tile([128, 1], f32, tag="rs")
                            if "fastmask" in opts:
                                nc.tensor.matmul(
                                    qk_ps, lhsT=q_v, rhs=k_v,
                                    start=True, stop=False)
                                nc.tensor.matmul(
                                    qk_ps, lhsT=ml_sb, rhs=mr_sb,
                                    start=False, stop=True)
                                nc.scalar.activation(
                                    att, qk_ps, Exp, bias=ebias, accum_out=rs)
                            else:
                                nc.tensor.matmul(
                                    qk_ps, lhsT=q_v, rhs=k_v,
                                    start=True, stop=True)
                                if "noatt_dve" not in opts:
                                    nc.vector.tensor_add(qk_ps, qk_ps, mask_sb)
                                nc.scalar.activation(att, qk_ps, Exp, accum_out=rs)
                            if "noatt_dve" not in opts:
                                if "divnorm" in opts:
                                    nc.vector.tensor_scalar(
                                        att, att, rs, None,
                                        op0=mybir.AluOpType.divide)
                                else:
                                    rc = attp.tile([128, 1], f32, tag="rc")
                                    nc.vector.reciprocal(rc, rs)
                                    nc.vector.tensor_scalar_mul(att, att, rc)

                            attT_ps = ps2.tile([128, 128], bf16, tag="attT")
                            nc.tensor.transpose(attT_ps, att, ident)
                            attT = attp.tile([128, 128], bf16, tag="attTs")
                            nc.vector.tensor_copy(attT, attT_ps)

                            if vphoist:
                                vp = v2_sb[:, g0, :]
                            else:
                                v_v = qkv_sb["v"][:, n0:n0 + 8, :].rearrange(
                                    "d g t -> d (g t)")
                                vp_ps = ps2.tile([128, 128], bf16, tag="vp")
                                nc.tensor.transpose(vp_ps, v_v, ident)
                                vp = attp.tile([128, 128], bf16, tag="vps")
                                nc.vector.tensor_copy(vp, vp_ps)

                            o_ps = ps2.tile(
                                [128, 128], f32, tag="o",
                                bufs=2 if vphoist else None)
                            nc.tensor.matmul(
                                o_ps, lhsT=vp, rhs=attT, start=True, stop=True)
                            if "scrsplit" in opts:
                                tw2 = TW // 2
                                half, tcl = tc_ // tw2, tc_ % tw2
                                dst = scr5[half][:, j0:j0 + 8, tcl:tcl + 1, :]
                            else:
                                dst = scr4[:, j0:j0 + 8, :, tc_:tc_ + 1]
                            nc.vector.tensor_copy(
                                dst, o_ps.rearrange("p (g h) -> p g h", g=8))

            # ---------------- phase 3: output projection ----------------
            wo_v = w_d["o"].ap().rearrange("(jt jp) r -> jp jt r", jp=128)
            out_v = out_d.ap().rearrange("(rt rp) n -> rp rt n", rp=128)
            if "scrsplit" in opts:
                ch3 = npos // 2
                chunks = [(scrA, slice(0, ch3)), (scrB, slice(ch3, npos))]
            else:
                ch3 = CH
                chunks = [
                    (scr_sb, slice(c * CH, (c + 1) * CH)) for c in range(NCH)]
            if fuse:
                _c3 = tc.tile_pool(name="ps3", bufs=2, space="PSUM")
                ps3_h = _c3.__enter__()
                held.append(_c3)
                ps3_ctx = contextlib.nullcontext(ps3_h)
            else:
                ps3_ctx = tc.tile_pool(name="ps3", bufs=3, space="PSUM")
            with ps3_ctx as ps3:
                for rg in range(8):
                    w_sb = wpool.tile([128, 16, 256], bf16, tag="w")
                    nc.sync.dma_start(w_sb, wo_v[:, :, rg * 256:(rg + 1) * 256])
                    for ro in range(2):
                        r = rg * 2 + ro
                        for src, csl in chunks:
                            ps = ps3.tile([128, ch3], f32)
                            for j in range(16):
                                rhs = (src[:, j, :] if "scrsplit" in opts
                                       else src[:, j, csl])
                                nc.tensor.matmul(
                                    ps,
                                    lhsT=w_sb[:, j, ro * 128:(ro + 1) * 128],
                                    rhs=rhs,
                                    start=(j == 0), stop=(j == 15))
                            o_sb = outp.tile([128, ch3], f32)
                            nc.scalar.activation(
                                o_sb, ps, Ident, bias=b_sb["o"][:, r:r + 1])
                            nc.sync.dma_start(out_v[:, r, csl], o_sb)

            for cm in reversed(held):
                cm.__exit__(None, None, None)

    def body_v2(tc):
        """Dependency-restructured pipeline:

        q-proj, k-proj (m-outer, weights streamed once) -> attention part1
        (qk+mask matmul, exp, normalize -> att_all in SBUF; needs only q,k)
        emitted BEFORE v-proj so part1's ACT/DVE work overlaps v's dense PE
        matmuls -> part2 (transposes + att@v + scatter) -> o-projection in
        two passes (scrA, scrB; Wo streamed twice) so the list scheduler
        fills part2 latency bubbles with o-proj matmuls.
        """
        with (
            tc.tile_pool(name="consts", bufs=1) as consts,
            tc.tile_pool(name="wpool", bufs=2) as wpool,
            tc.tile_pool(name="vpool", bufs=1) as vpool,
            tc.tile_pool(name="attall", bufs=1) as attall,
            tc.tile_pool(name="attp", bufs=3) as attp,
            tc.tile_pool(name="tmp", bufs=2) as tmp,
            tc.tile_pool(name="outp", bufs=2) as outp,
        ):
            cos_sb = consts.tile([128, npos], f32)
            nc.sync.dma_start(cos_sb, cos_d.ap())
            sin_sb = consts.tile([128, npos], f32)
            nc.sync.dma_start(sin_sb, sin_d.ap())
            if "dvemask" in opts:
                mask4 = consts.tile([128, 4, 128], f32)
                for qi in range(4):
                    nc.sync.dma_start(mask4[:, qi, :], mask_d.ap())
            else:
                ml_sb = consts.tile([8, 128], bf16)
                nc.sync.dma_start(ml_sb, maskl_d.ap())
                mr4_sb = consts.tile([8, 512], bf16)
                nc.sync.dma_start(mr4_sb, maskr4_d.ap())
                ebias = consts.tile([128, 1], f32)
                nc.vector.memset(ebias, -MASKVAL)
            ident = consts.tile([128, 128], bf16)
            make_identity(nc, ident)
            b_sb = {}
            for p in ("q", "k", "v", "o"):
                b_sb[p] = consts.tile([128, 16], f32, tag=f"b_{p}", name=f"b_{p}")
                nc.sync.dma_start(b_sb[p], b_d[p].ap().rearrange("(t p) -> p t", p=128))

            att_all = attall.tile([128, NQ // 4, 4, 128], bf16)
            v_sb = vpool.tile([128, npos, 16], bf16, tag="qkv_v", name="qkv_v")

            with (
                tc.tile_pool(name="qkp", bufs=1) as qkp,
                tc.tile_pool(name="xp", bufs=1) as xp,
                tc.tile_pool(
                    name="ps1", bufs=4 if "wreuse" in opts else 3,
                    space="PSUM") as ps1,
                tc.tile_pool(name="psqk", bufs=3, space="PSUM") as psqk,
            ):
                xt_sb = xp.tile([128, 16, npos], bf16)
                xt_v = xt_d.ap().rearrange("(kt kp) n -> kp kt n", kp=128)
                nxc = max(NCH, 4)
                xcw = npos // nxc
                for xc in range(nxc):
                    csl = slice(xc * xcw, (xc + 1) * xcw)
                    nc.sync.dma_start(xt_sb[:, :, csl], xt_v[:, :, csl])

                qk_sb = {
                    p: qkp.tile([128, npos, 16], bf16,
                                tag=f"qkv_{p}", name=f"qkv_{p}")
                    for p in ("q", "k")
                }

                wreuse = "wreuse" in opts

                nobias = "nobias" in opts

                def evict(p, dst, ps, m, csl):
                    qdst = dst[:, csl, m]
                    if p == "v" or "norope" in opts:
                        if nobias:
                            nc.scalar.copy(qdst, ps)
                        else:
                            nc.scalar.activation(
                                qdst, ps, Ident, bias=b_sb[p][:, m:m + 1])
                    else:
                        if not nobias:
                            nc.vector.tensor_scalar_add(
                                ps, ps, b_sb[p][:, m:m + 1])
                        t1 = tmp.tile([128, CH], f32, tag="t1")
                        nc.vector.tensor_mul(t1, ps, cos_sb[:, csl])
                        tsw = tmp.tile([128, CH], f32, tag="tsw")
                        nc.scalar.copy(tsw[0:64, :], ps[64:128, :])
                        nc.scalar.copy(tsw[64:128, :], ps[0:64, :])
                        nc.vector.tensor_mul(tsw, tsw, sin_sb[:, csl])
                        nc.vector.tensor_add(qdst, t1, tsw)

                wlay = "wlay" in opts

                def proj(p, dst):
                    wv_d = w_d[p].ap().rearrange("(kt kp) e -> kp kt e", kp=128)
                    for mg in range(8):
                        w_sb = wpool.tile([128, 16, 256], bf16, tag="w")
                        if wlay:
                            nc.sync.dma_start(w_sb, wr_d[p].ap()[mg])
                        else:
                            nc.sync.dma_start(
                                w_sb, wv_d[:, :, mg * 256:(mg + 1) * 256])
                        for mo in range(2):
                            m = mg * 2 + mo
                            if wreuse:
                                # kt-outer: consecutive matmuls share the
                                # stationary operand (one weight load per
                                # kt feeds both column chunks)
                                pss = [
                                    ps1.tile([128, CH], f32, tag="ps1",
                                             name=f"ps1_{ch}")
                                    for ch in range(NCH)
                                ]
                                for kt in range(16):
                                    for ch in range(NCH):
                                        nc.tensor.matmul(
                                            pss[ch],
                                            lhsT=w_sb[:, kt, mo * 128:(mo + 1) * 128],
                                            rhs=xt_sb[:, kt, ch * CH:(ch + 1) * CH],
                                            start=(kt == 0), stop=(kt == 15))
                                for ch in range(NCH):
                                    evict(p, dst, pss[ch], m,
                                          slice(ch * CH, (ch + 1) * CH))
                            else:
                                for ch in range(NCH):
                                    csl = slice(ch * CH, (ch + 1) * CH)
                                    ps = ps1.tile([128, CH], f32, tag="ps1")
                                    for kt in range(16):
                                        nc.tensor.matmul(
                                            ps,
                                            lhsT=w_sb[:, kt, mo * 128:(mo + 1) * 128],
                                            rhs=xt_sb[:, kt, csl],
                                            start=(kt == 0), stop=(kt == 15))
                                    evict(p, dst, ps, m, csl)

                proj("q", qk_sb["q"])
                proj("k", qk_sb["k"])

                # attention part1: qk + mask -> exp -> normalize -> att_all
                for a in range(0 if "projonly" in opts else NQ // 4):
                    qk4 = psqk.tile([128, 4, 128], f32, tag="qk4")
                    for qi in range(4):
                        n0 = 32 * a + 8 * qi
                        q_v = qk_sb["q"][:, n0:n0 + 8, :].rearrange(
                            "d g h -> d (g h)")
                        k_v = qk_sb["k"][:, n0:n0 + 8, :].rearrange(
                            "d g h -> d (g h)")
                        nc.tensor.matmul(
                            qk4[:, qi, :], lhsT=q_v, rhs=k_v,
                            start=(qi == 0),
                            stop=("dvemask" in opts and qi == 3),
                            skip_group_check=True)
                    att4 = att_all[:, a, :, :]
                    if "dvemask" in opts:
                        nc.vector.tensor_add(qk4, qk4, mask4)
                        nc.scalar.activation(att4, qk4, Exp)
                    else:
                        nc.tensor.matmul(
                            qk4.rearrange("p q n -> p (q n)"),
                            lhsT=ml_sb, rhs=mr4_sb,
                            start=False, stop=True, skip_group_check=True)
                        nc.scalar.activation(att4, qk4, Exp, bias=ebias)
                    rs4 = attp.tile([128, 4], f32, tag="rs4")
                    nc.vector.reduce_sum(
                        out=rs4, in_=att4, axis=mybir.AxisListType.X)
                    rc4 = attp.tile([128, 4], f32, tag="rc4")
                    nc.vector.reciprocal(rc4, rs4)
                    for qi in range(4):
                        nc.vector.tensor_scalar_mul(
                            att4[:, qi, :], att4[:, qi, :], rc4[:, qi:qi + 1])

                proj("v", v_sb)

            # ---- part2 + o-projection, interleaved by the scheduler ----
            with (
                tc.tile_pool(name="scrp", bufs=1) as scrp,
                tc.tile_pool(name="ps2", bufs=2, space="PSUM") as ps2,
                tc.tile_pool(name="ps3", bufs=4, space="PSUM") as ps3,
            ):
                tw2 = TW // 2
                scrA = scrp.tile([128, 16, npos // 2], bf16, tag="scrA")
                scrB = scrp.tile([128, 16, npos // 2], bf16, tag="scrB")
                scr5 = [
                    s.rearrange("p j (t h) -> p j t h", h=16)
                    for s in (scrA, scrB)
                ]

                def part2(a):
                    tps_ps = ps2.tile([128, 8, 128], bf16, tag="tps")
                    for qi in range(4):
                        n0 = 32 * a + 8 * qi
                        v_v = v_sb[:, n0:n0 + 8, :].rearrange(
                            "d g t -> d (g t)")
                        nc.tensor.matmul(
                            tps_ps[:, 4 + qi, :], lhsT=v_v,
                            rhs=ident, is_transpose=True,
                            start=(qi == 0), stop=False,
                            skip_group_check=True)
                    for qi in range(4):
                        nc.tensor.matmul(
                            tps_ps[:, qi, :],
                            lhsT=att_all[:, a, qi, :], rhs=ident,
                            is_transpose=True,
                            start=False, stop=(qi == 3),
                            skip_group_check=True)
                    tps = attp.tile([128, 8, 128], bf16, tag="tpss")
                    nc.scalar.copy(tps, tps_ps)
                    o4_ps = ps2.tile([128, 4, 128], f32, tag="o4")
                    for qi in range(4):
                        nc.tensor.matmul(
                            o4_ps[:, qi, :],
                            lhsT=tps[:, 4 + qi, :],
                            rhs=tps[:, qi, :],
                            start=(qi == 0), stop=(qi == 3),
                            skip_group_check=True)
                    tc0 = 2 * a
                    half, tcl0 = tc0 // tw2, tc0 % tw2
                    dst = scr5[half][:, :, tcl0:tcl0 + 2, :].rearrange(
                        "p (jb g) t h -> p t jb g h", jb=2)
                    nc.vector.tensor_copy(
                        dst,
                        o4_ps.rearrange(
                            "p (tb jb) (g h) -> p tb jb g h",
                            jb=2, h=16))

                wo_v = w_d["o"].ap().rearrange("(jt jp) r -> jp jt r", jp=128)
                out_v = out_d.ap().rearrange("(rt rp) n -> rp rt n", rp=128)

                def oproj_pass(src, osl):
                    for rg in range(8):
                        w_sb = wpool.tile([128, 16, 256], bf16, tag="w")
                        if "wlay" in opts:
                            nc.sync.dma_start(w_sb, wr_d["o"].ap()[rg])
                        else:
                            nc.sync.dma_start(
                                w_sb, wo_v[:, :, rg * 256:(rg + 1) * 256])
                        for ro in range(2):
                            r = rg * 2 + ro
                            ps = ps3.tile([128, npos // 2], f32, tag="ps3")
                            for j in range(16):
                                nc.tensor.matmul(
                                    ps,
                                    lhsT=w_sb[:, j, ro * 128:(ro + 1) * 128],
                                    rhs=src[:, j, :],
                                    start=(j == 0), stop=(j == 15))
                            o_sb = outp.tile([128, npos // 2], out_dt)
                            if "nobias" in opts:
                                nc.scalar.copy(o_sb, ps)
                            else:
                                nc.scalar.activation(
                                    o_sb, ps, Ident, bias=b_sb["o"][:, r:r + 1])
                            nc.sync.dma_start(out_v[:, r, osl], o_sb)

                if "projonly" in opts:
                    nc.vector.memset(scrA, 0.0)
                    nc.vector.memset(scrB, 0.0)
                    oproj_pass(scrA, slice(0, npos // 2))
                    oproj_pass(scrB, slice(npos // 2, npos))
                else:
                    for a in range(NQ // 8):
                        part2(a)
                    oproj_pass(scrA, slice(0, npos // 2))
                    for a in range(NQ // 8, NQ // 4):
                        part2(a)
                    oproj_pass(scrB, slice(npos // 2, npos))

    with tile.TileContext(nc) as tc:
        for _ in range(reps):
            if "v2" in opts:
                body_v2(tc)
            else:
                body(tc)

    nc.compile()
    _NC_CACHE[key] = nc
    return nc


# ---------------------------------------------------------------------------
# Runner (PJRT via axon, cached jitted callable)
# ---------------------------------------------------------------------------

_RUNNER_CACHE = {}


def make_runner(nc, ncores=NCORES):
    """Returns run(in_maps) -> list of per-core output dicts.

    Mirrors bass2jax.run_bass_via_pjrt but caches the jitted callable and
    does NOT donate output buffers (kernel writes every output element), so
    repeated timed calls don't re-trace or re-transfer.
    """
    key = id(nc)
    if key in _RUNNER_CACHE:
        return _RUNNER_CACHE[key]

    import jax
    import numpy as _np
    from jax.sharding import Mesh, PartitionSpec
    from jax.experimental.shard_map import shard_map
    from concourse import mybir
    from concourse import bass2jax
    from concourse.bass2jax import (
        _bass_exec_p, install_neuronx_cc_hook, partition_id_tensor)

    install_neuronx_cc_hook()

    partition_name = (
        nc.partition_id_tensor.name if nc.partition_id_tensor else None)
    in_names, out_names, out_avals, zero_outs = [], [], [], []
    for alloc in nc.m.functions[0].allocations:
        if not isinstance(alloc, mybir.MemoryLocationSet):
            continue
        name = alloc.memorylocations[0].name
        if alloc.kind == "ExternalInput":
            if name == partition_name:
                continue
            in_names.append(name)
        elif alloc.kind == "ExternalOutput":
            shape = tuple(alloc.tensor_shape)
            dtype = mybir.dt.np(alloc.dtype)
            out_names.append(name)
            out_avals.append(jax.core.ShapedArray(shape, dtype))
            zero_outs.append(_np.zeros(shape, dtype))
    n_params = len(in_names)
    all_in_names = in_names + out_names
    if partition_name is not None:
        all_in_names = all_in_names + [partition_name]

    def _body(*args):
        operands = list(args)
        if partition_name is not None:
            operands.append(partition_id_tensor())
        outs = _bass_exec_p.bind(
            *operands,
            out_avals=tuple(out_avals),
            in_names=tuple(all_in_names),
            out_names=tuple(out_names),
            lowering_input_output_aliases=(),
            sim_require_finite=True,
            sim_require_nnan=True,
            nc=nc,
        )
        return tuple(outs)

    devices = jax.devices()[:ncores]
    mesh = Mesh(np.asarray(devices), ("core",))
    n_outs = len(out_names)
    jitted = jax.jit(
        shard_map(
            _body, mesh=mesh,
            in_specs=(PartitionSpec("core"),) * (n_params + n_outs),
            out_specs=(PartitionSpec("core"),) * n_outs,
            check_rep=False,
        ),
        keep_unused=True,
    )

    zeros_dev = [
        jax.device_put(
            _np.zeros((ncores * z.shape[0], *z.shape[1:]), z.dtype))
        for z in zero_outs
    ]

    def put(in_maps):
        concat = [
            _np.concatenate([_np.asarray(m[name]) for m in in_maps], axis=0)
            for name in in_names
        ]
        return [jax.device_put(a) for a in concat]

    def run_dev(in_dev):
        outs = jitted(*in_dev, *zeros_dev)
        jax.block_until_ready(outs)
        return outs

    def run(in_maps):
        outs = run_dev(put(in_maps))
        res = []
        for c in range(len(in_maps)):
            res.append({
                name: _np.asarray(outs[i]).reshape(
                    len(in_maps), *out_avals[i].shape)[c]
                for i, name in enumerate(out_names)
            })
        return res

    run.put = put
    run.run_dev = run_dev
    run.out_names = out_names
    _RUNNER_CACHE[key] = run
    return run


DEFAULT_OPTS = frozenset({"v2", "dvemask", "wlay", "obf16"})


def kernel(**inputs) -> np.ndarray:
    inputs = {k: np.asarray(v) for k, v in inputs.items()}
    opts = set(DEFAULT_OPTS)
    if all(not np.any(inputs[b]) for b in ("bq", "bk", "bv", "bo")):
        opts.add("nobias")
    opts = frozenset(opts)
    in_maps, meta = prepare_host(**inputs)
    nc = build_nc(NPOS, opts=opts)
    run = make_runner(nc, NCORES)
    outs = run(in_maps)
    layout = "t_h" if ("scrsplit" in opts or "v2" in opts) else "h_t"
    return assemble_output(outs, meta, NPOS, layout)



# revision 29
# speedup vs baseline: 2.9904x; 2.9198x over previous
"""Trainium2 Bass kernel for nn_MultiHeadAttention_81664508166458.

Reference computes a "cross-head" MHA: per (batch, position) the attention
matrix is HxH (H=16 heads), contracting head_dim D=128. Every position is
independent, so we shard the 8192 (batch, position) pairs across 8 cores
(1024 each), fully data-parallel, no collectives.

Host-side preprocessing (part of sharding, not timed device work):
  - weights transposed to [e_in, e_out] (k-major) and cast to bf16
  - RoPE pair permutation baked into Wq/Wk rows: head-local dim d' with
    x0 (even d) in d'=[0,64) and x1 (odd d) in d'=[64,128) so the rotation
    becomes same-partition table multiplies plus a half-swap
  - 1/sqrt(D) attention scale baked into Wq/bq
  - x transposed to [e_in, n] bf16
  - cos/sin tables and the block-diagonal softmax mask precomputed

Device pipeline per core (all matmuls bf16 with fp32 PSUM accumulation;
"v2" dependency-restructured flow, chosen so the Tile list scheduler keeps
the PE near-saturated — PE work is ~1.13M cycles and is the binding
resource):
  1. q-proj, k-proj [d, n, h] = W*T.T @ xT (16 e-tiles x 16 k-tiles,
     N=512), RoPE fused into the PSUM eviction (DVE table mults + ACT
     half-swap)
  2. attention part1 (only needs q,k — emitted BEFORE the v projection so
     its exp/normalize ACT+DVE work overlaps v's dense PE matmuls): per
     4-quad group qk+mask matmuls -> exp -> row-sum -> normalize, with
     normalized att kept in a persistent 4MB SBUF tile
  3. v-proj (dense PE, runs under part1's ACT/DVE tail)
  4. part2 per group: PE-transpose att and v slices, O^T = vp @ attT,
     DVE-scatter into scr; emitted as [A-half, o-proj pass A, B-half,
     o-proj pass B] so o-proj matmuls fill part2's latency bubbles
  5. outT[r, (t,h)] = WoT.T @ scr (+bo) in two passes (Wo streamed twice),
     DMA to DRAM [E, n] (host transposes)
"""

import numpy as np
import ml_dtypes

B, S, E = 4, 2048, 2048
H, D = 16, 128
NCORES = 8
CORES_PER_BATCH = NCORES // B          # 2
NPOS = S // CORES_PER_BATCH            # 1024 positions per core
THETA = 10000.0
MASK_NEG = -30000.0

BF16 = ml_dtypes.bfloat16

# ---------------------------------------------------------------------------
# Host-side preprocessing
# ---------------------------------------------------------------------------


def _rope_perm():
    """P_IDX[new] = old row index: x0 (even d) -> d'=[0,64), x1 (odd) -> [64,128)."""
    p = np.empty(E, np.int64)
    for h in range(H):
        base = h * D
        i = np.arange(D // 2)
        p[base + i] = base + 2 * i
        p[base + 64 + i] = base + 2 * i + 1
    return p


def _rope_tables(npos, offset):
    """cos table C[p, n] and signed sin table S[p, n], p in [0,128)."""
    inv = 1.0 / (THETA ** (np.arange(0, D, 2, dtype=np.float64) / D))  # [64]
    pos = np.arange(offset, offset + npos, dtype=np.float64)
    fr = np.outer(inv, pos)  # [64, npos]
    c = np.cos(fr).astype(np.float32)
    s = np.sin(fr).astype(np.float32)
    cos_b = np.concatenate([c, c], axis=0)            # [128, npos]
    sin_b = np.concatenate([-s, s], axis=0)           # signed
    return np.ascontiguousarray(cos_b), np.ascontiguousarray(sin_b)


def _blockdiag_mask():
    m = np.full((128, 128), MASK_NEG, np.float32)
    for g in range(8):
        m[g * 16:(g + 1) * 16, g * 16:(g + 1) * 16] = 0.0
    return m


# exact bf16-representable mask magnitude (softmax is shift-invariant, but we
# keep the on-diagonal shift exactly zero: +MASKVAL via matmul, -MASKVAL bias)
MASKVAL = float(np.float32(BF16(30000.0)))


def _mask_mm():
    """K=8 rank-8 matmul operands adding +MASKVAL on the block diagonal.
    maskl[g, p] = MASKVAL if p//16==g else 0 ; maskr[g, f] = 1 if f//16==g."""
    ind = np.zeros((8, 128), np.float32)
    for g in range(8):
        ind[g, g * 16:(g + 1) * 16] = 1.0
    return (ind * MASKVAL).astype(BF16), ind.astype(BF16)


def _repack_w(wt):
    """[E_in=(kt kp), E_out=(mg e)] -> [mg, kp, kt, e] with e=256."""
    w4 = np.asarray(wt).reshape(16, 128, 8, 256)      # kt kp mg e
    return np.ascontiguousarray(np.transpose(w4, (2, 1, 0, 3)))


def prepare_host(x, Wq, bq, Wk, bk, Wv, bv, Wo, bo, npos=NPOS, ncores=NCORES):
    """Returns (shared weight arrays dict, list of per-core in_maps)."""
    x = np.asarray(x, np.float32)
    perm = _rope_perm()
    scale = np.float32(1.0 / np.sqrt(D))

    wqt = np.ascontiguousarray((np.asarray(Wq, np.float32)[perm, :] * scale).T).astype(BF16)
    wkt = np.ascontiguousarray(np.asarray(Wk, np.float32)[perm, :].T).astype(BF16)
    wvt = np.ascontiguousarray(np.asarray(Wv, np.float32).T).astype(BF16)
    wot = np.ascontiguousarray(np.asarray(Wo, np.float32).T).astype(BF16)
    bq_p = (np.asarray(bq, np.float32)[perm] * scale).copy()
    bk_p = np.asarray(bk, np.float32)[perm].copy()
    bv_p = np.asarray(bv, np.float32).copy()
    bo_p = np.asarray(bo, np.float32).copy()
    mask = _blockdiag_mask()
    maskl, maskr = _mask_mm()

    in_maps = []
    meta = []
    for c in range(ncores):
        bc = c // CORES_PER_BATCH
        o = (c % CORES_PER_BATCH) * npos
        xc = x[bc, o:o + npos, :]                      # [npos, E]
        xt = np.ascontiguousarray(xc.T).astype(BF16)   # [E, npos]
        cos_b, sin_b = _rope_tables(npos, o)
        in_maps.append({
            "xt": xt, "wqt": wqt, "wkt": wkt, "wvt": wvt, "wot": wot,
            "bq": bq_p, "bk": bk_p, "bv": bv_p, "bo": bo_p,
            "cosb": cos_b, "sinb": sin_b, "mask": mask,
            "maskl": maskl, "maskr": maskr,
            "maskr4": np.ascontiguousarray(np.tile(maskr, (1, 4))),
            # weights repacked [mg, kp, kt, e256] so each per-partition DMA
            # row is 16*256*2 = 8KB contiguous (full DMA line efficiency)
            "wqtr": _repack_w(wqt), "wktr": _repack_w(wkt),
            "wvtr": _repack_w(wvt), "wotr": _repack_w(wot),
        })
        meta.append((bc, o))
    return in_maps, meta


def assemble_output(outs, meta, npos=NPOS, layout="h_t"):
    """outs: list of per-core {'outt': [E, npos] f32}. Returns [B, S, E].

    layout "h_t": outt col = h*tw + tc (tc local).
    layout "t_h": outt col = tc*16 + h (scrsplit build).
    """
    full = np.empty((B, S, E), np.float32)
    tw = npos // 16
    for (bc, o), res in zip(meta, outs):
        outt = res["outt"]
        if layout == "h_t":
            v = outt.reshape(E, H, tw)           # [E, h, tc]
            v = np.transpose(v, (1, 2, 0))       # [h, tc, E]
        else:
            v = outt.reshape(E, tw, H)           # [E, tc, h]
            v = np.transpose(v, (2, 1, 0))       # [h, tc, E]
        t0 = o // 16
        for h in range(H):
            full[bc, h * 128 + t0: h * 128 + t0 + tw, :] = v[h]
    return full


# ---------------------------------------------------------------------------
# Numpy emulator of the exact device dataflow (index-math validation)
# ---------------------------------------------------------------------------


def emulate_core(im, npos=NPOS, layout="h_t"):
    f32 = np.float32
    xt = im["xt"].astype(f32)
    qT = (im["wqt"].astype(f32).T @ xt) + im["bq"][:, None]   # [E, n]
    kT = (im["wkt"].astype(f32).T @ xt) + im["bk"][:, None]
    vT = (im["wvt"].astype(f32).T @ xt) + im["bv"][:, None]
    C, Sg = im["cosb"].astype(f32), im["sinb"].astype(f32)

    def rope(t):
        t3 = t.reshape(H, D, npos)                            # [h, d', n]
        sw = np.concatenate([t3[:, 64:, :], t3[:, :64, :]], axis=1)
        r = t3 * C[None] + sw * Sg[None]
        return r.astype(BF16).astype(f32)

    qr, kr = rope(qT), rope(kT)
    vb = vT.astype(BF16).astype(f32).reshape(H, D, npos)
    scr = np.zeros((D, 16, npos), f32)                        # [d, j, h*tw+tc]
    tw = npos // 16
    for g0 in range(npos // 8):
        n0 = 8 * g0
        j0, tc = n0 % 16, g0 // 2
        q_blk = qr[:, :, n0:n0 + 8]                           # [h, d, g]
        k_blk = kr[:, :, n0:n0 + 8]
        lhs = np.transpose(q_blk, (1, 2, 0)).reshape(D, 128)  # [d, (g,h)]
        rhs = np.transpose(k_blk, (1, 2, 0)).reshape(D, 128)  # [d, (g,t)]
        qk = lhs.T @ rhs + im["mask"]
        e = np.exp(qk)
        att = (e / e.sum(1, keepdims=True)).astype(BF16).astype(f32)
        vm = np.transpose(vb[:, :, n0:n0 + 8], (1, 2, 0)).reshape(D, 128)  # [d,(g,t)]
        # out2T[d, (g,h)] = sum_{(g,t)} vm[d, (g,t)] * att[(g,h), (g,t)]
        o2 = vm @ att.T                 # [d, (g,h)]
        o2v = o2.reshape(D, 8, 16)
        if layout == "h_t":
            scr.reshape(D, 16, H, tw)[:, j0:j0 + 8, :, tc] = o2v
        else:
            scr.reshape(D, 16, tw, H)[:, j0:j0 + 8, tc, :] = o2v
    # scr[d, j, col] -> rhs row e=(j*128+d)
    rhs_full = np.transpose(scr, (1, 0, 2)).reshape(16 * D, npos).astype(BF16).astype(f32)
    outt = im["wot"].astype(f32).T @ rhs_full + im["bo"][:, None]
    return {"outt": outt.astype(f32)}


def emulate_full(inputs, npos=NPOS, ncores=NCORES, layout="h_t"):
    in_maps, meta = prepare_host(**inputs, npos=npos, ncores=ncores)
    outs = [emulate_core(im, npos, layout) for im in in_maps]
    return assemble_output(outs, meta, npos, layout)


# ---------------------------------------------------------------------------
# Bass kernel
# ---------------------------------------------------------------------------

_NC_CACHE = {}


def build_nc(npos=NPOS, reps=1, opts=frozenset()):
    import concourse.bass as bass
    import concourse.tile as tile
    from concourse import bacc, mybir
    from concourse.masks import make_identity

    opts = frozenset(opts)
    key = (npos, reps, opts)
    if key in _NC_CACHE:
        return _NC_CACHE[key]

    f32, bf16 = mybir.dt.float32, mybir.dt.bfloat16
    CH = 256 if "ch256" in opts else min(512, npos)  # free-dim chunk
    NCH = npos // CH
    TW = npos // 16
    NQ = npos // 8               # number of 8-position quads

    nc = bacc.Bacc("TRN2", target_bir_lowering=False, debug=False)

    xt_d = nc.dram_tensor("xt", [E, npos], bf16, kind="ExternalInput")
    w_d = {
        "q": nc.dram_tensor("wqt", [E, E], bf16, kind="ExternalInput"),
        "k": nc.dram_tensor("wkt", [E, E], bf16, kind="ExternalInput"),
        "v": nc.dram_tensor("wvt", [E, E], bf16, kind="ExternalInput"),
        "o": nc.dram_tensor("wot", [E, E], bf16, kind="ExternalInput"),
    }
    wr_d = {
        "q": nc.dram_tensor("wqtr", [8, 128, 16, 256], bf16, kind="ExternalInput"),
        "k": nc.dram_tensor("wktr", [8, 128, 16, 256], bf16, kind="ExternalInput"),
        "v": nc.dram_tensor("wvtr", [8, 128, 16, 256], bf16, kind="ExternalInput"),
        "o": nc.dram_tensor("wotr", [8, 128, 16, 256], bf16, kind="ExternalInput"),
    }
    b_d = {
        "q": nc.dram_tensor("bq", [E], f32, kind="ExternalInput"),
        "k": nc.dram_tensor("bk", [E], f32, kind="ExternalInput"),
        "v": nc.dram_tensor("bv", [E], f32, kind="ExternalInput"),
        "o": nc.dram_tensor("bo", [E], f32, kind="ExternalInput"),
    }
    cos_d = nc.dram_tensor("cosb", [128, npos], f32, kind="ExternalInput")
    sin_d = nc.dram_tensor("sinb", [128, npos], f32, kind="ExternalInput")
    mask_d = nc.dram_tensor("mask", [128, 128], f32, kind="ExternalInput")
    maskl_d = nc.dram_tensor("maskl", [8, 128], bf16, kind="ExternalInput")
    maskr_d = nc.dram_tensor("maskr", [8, 128], bf16, kind="ExternalInput")
    maskr4_d = nc.dram_tensor("maskr4", [8, 512], bf16, kind="ExternalInput")
    out_dt = bf16 if "obf16" in opts else f32
    out_d = nc.dram_tensor("outt", [E, npos], out_dt, kind="ExternalOutput")

    Exp = mybir.ActivationFunctionType.Exp
    Ident = mybir.ActivationFunctionType.Identity

    def body(tc):
        with (
            tc.tile_pool(name="consts", bufs=1) as consts,
            tc.tile_pool(name="wpool", bufs=2) as wpool,
            tc.tile_pool(name="scrp", bufs=1) as scrp,
            tc.tile_pool(
                name="tmp", bufs=3 if "tmpb3" in opts else 2) as tmp,
            tc.tile_pool(
                name="attp", bufs=4 if "attb4" in opts else 3) as attp,
            tc.tile_pool(
                name="outp", bufs=3 if "outb3" in opts else 2) as outp,
        ):
            cos_sb = consts.tile([128, npos], f32)
            nc.sync.dma_start(cos_sb, cos_d.ap())
            sin_sb = consts.tile([128, npos], f32)
            nc.sync.dma_start(sin_sb, sin_d.ap())
            if "fastmask" in opts:
                ml_sb = consts.tile([8, 128], bf16)
                nc.sync.dma_start(ml_sb, maskl_d.ap())
                mr_sb = consts.tile([8, 128], bf16)
                nc.sync.dma_start(mr_sb, maskr_d.ap())
                ebias = consts.tile([128, 1], f32)
                nc.vector.memset(ebias, -MASKVAL)
                if "maskw" in opts:
                    mr4_sb = consts.tile([8, 512], bf16)
                    nc.sync.dma_start(mr4_sb, maskr4_d.ap())
            else:
                mask_sb = consts.tile([128, 128], f32)
                nc.sync.dma_start(mask_sb, mask_d.ap())
            ident = consts.tile([128, 128], bf16)
            make_identity(nc, ident)
            b_sb = {}
            for p in ("q", "k", "v", "o"):
                b_sb[p] = consts.tile([128, 16], f32, tag=f"b_{p}", name=f"b_{p}")
                nc.sync.dma_start(b_sb[p], b_d[p].ap().rearrange("(t p) -> p t", p=128))

            if "scrsplit" in opts:
                scrA = scrp.tile([128, 16, npos // 2], bf16, tag="scrA")
                scrB = scrp.tile([128, 16, npos // 2], bf16, tag="scrB")
            else:
                scr_sb = scrp.tile([128, 16, npos], bf16)

            with tc.tile_pool(name="qkvp", bufs=1) as qkvp:
                # layout [d, n, h]: per-quad (g,h)/(g,t) views are contiguous
                if "chouter" in opts:
                    # per-chunk tiles so attention can start once a chunk's
                    # projections finish (tile-granular RAW deps)
                    qkv_ch = {
                        p: [
                            qkvp.tile([128, CH, 16], bf16,
                                      tag=f"qkv_{p}{c}", name=f"qkv_{p}{c}")
                            for c in range(NCH)
                        ]
                        for p in ("q", "k", "v")
                    }
                else:
                    qkv_sb = {
                        p: qkvp.tile([128, npos, 16], bf16,
                                     tag=f"qkv_{p}", name=f"qkv_{p}")
                        for p in ("q", "k", "v")
                    }

                # ---------------- phase 1: projections ----------------
                import contextlib
                fuse = "fuse" in opts
                held = []
                xp_ctx = tc.tile_pool(name="xp", bufs=1)
                if fuse:
                    # keep all PSUM pools open across phases (2+4+2=8 banks)
                    # so the scheduler can fill attention-chain PE stalls
                    # with projection/O-proj matmuls
                    ps1_cm = tc.tile_pool(name="ps1", bufs=2, space="PSUM")
                    ps1_h = ps1_cm.__enter__()
                    held.append(ps1_cm)
                    ps1_ctx = contextlib.nullcontext(ps1_h)
                else:
                    ps1_ctx = tc.tile_pool(
                        name="ps1", bufs=4 if "ps1b4" in opts else 3,
                        space="PSUM")
                with (xp_ctx as xp, ps1_ctx as ps1):
                    xt_sb = xp.tile([128, 16, npos], bf16)
                    nc.sync.dma_start(
                        xt_sb, xt_d.ap().rearrange("(kt kp) n -> kp kt n", kp=128))

                    if "chouter" in opts:
                        loop_iter = [
                            (p, ch, mg)
                            for p in ("q", "k", "v")
                            for ch in range(NCH)
                            for mg in range(8)
                        ]
                    else:
                        loop_iter = [
                            (p, None, mg)
                            for p in ("q", "k", "v")
                            for mg in range(8)
                        ]
                    for p, ch_o, mg in loop_iter:
                        wv_d = w_d[p].ap().rearrange("(kt kp) e -> kp kt e", kp=128)
                        if True:
                            w_sb = wpool.tile([128, 16, 256], bf16, tag="w")
                            nc.sync.dma_start(
                                w_sb, wv_d[:, :, mg * 256:(mg + 1) * 256])
                            for mo in range(2):
                                m = mg * 2 + mo
                                for ch in ([ch_o] if ch_o is not None
                                           else range(NCH)):
                                    csl = slice(ch * CH, (ch + 1) * CH)
                                    ps = ps1.tile([128, CH], f32)
                                    for kt in range(16):
                                        nc.tensor.matmul(
                                            ps,
                                            lhsT=w_sb[:, kt, mo * 128:(mo + 1) * 128],
                                            rhs=xt_sb[:, kt, csl],
                                            start=(kt == 0), stop=(kt == 15))
                                    if "chouter" in opts:
                                        qdst = qkv_ch[p][ch][:, :, m]
                                    else:
                                        qdst = qkv_sb[p][:, csl, m]
                                    if p == "v" or "norope" in opts:
                                        nc.scalar.activation(
                                            qdst, ps, Ident,
                                            bias=b_sb["v"][:, m:m + 1])
                                    else:
                                        nc.vector.tensor_scalar_add(
                                            ps, ps, b_sb[p][:, m:m + 1])
                                        t1 = tmp.tile([128, CH], f32, tag="t1")
                                        nc.vector.tensor_mul(t1, ps, cos_sb[:, csl])
                                        tsw = tmp.tile([128, CH], f32, tag="tsw")
                                        nc.scalar.copy(tsw[0:64, :], ps[64:128, :])
                                        nc.scalar.copy(tsw[64:128, :], ps[0:64, :])
                                        nc.vector.tensor_mul(tsw, tsw, sin_sb[:, csl])
                                        nc.vector.tensor_add(qdst, t1, tsw)

                # ---------------- phase 2: attention ----------------
                if "scrsplit" in opts:
                    scr5 = [
                        s.rearrange("p j (t h) -> p j t h", h=16)
                        for s in (scrA, scrB)
                    ]
                else:
                    scr4 = scr_sb.rearrange("p j (h t) -> p j h t", h=16)
                nquads = npos // 8
                vphoist = "vphoist" in opts

                with tc.tile_pool(name="v2p", bufs=1) as v2p:
                    if vphoist:
                        v2_sb = v2p.tile([128, nquads, 128], bf16)
                        with tc.tile_pool(
                                name="vpps", bufs=4, space="PSUM") as vpps:
                            for g0 in range(nquads):
                                n0 = 8 * g0
                                v_v = qkv_sb["v"][:, n0:n0 + 8, :].rearrange(
                                    "d g t -> d (g t)")
                                vp_ps = vpps.tile([128, 128], bf16, tag="vp")
                                nc.tensor.transpose(vp_ps, v_v, ident)
                                nc.vector.tensor_copy(v2_sb[:, g0, :], vp_ps)

                    if fuse:
                        _c = tc.tile_pool(name="ps2", bufs=1, space="PSUM")
                        ps2_h = _c.__enter__()
                        held.append(_c)
                        ps2_cm = contextlib.nullcontext(ps2_h)
                    elif vphoist:
                        ps2_cm = tc.tile_pool(name="ps2", bufs=3, space="PSUM")
                    else:
                        ps2_cm = tc.tile_pool(name="ps2", bufs=2, space="PSUM")
                    with ps2_cm as ps2:
                        if "noatt" in opts:
                            if "scrsplit" in opts:
                                nc.vector.memset(scrA, 0.0)
                                nc.vector.memset(scrB, 0.0)
                            else:
                                nc.vector.memset(scr_sb, 0.0)
                        if "qbatch" in opts:
                            assert {"fastmask", "scrsplit"} <= opts
                            tw2 = TW // 2

                            def qkv_slice(p, n0):
                                if "chouter" in opts:
                                    return qkv_ch[p][n0 // CH][
                                        :, n0 % CH:n0 % CH + 8, :]
                                return qkv_sb[p][:, n0:n0 + 8, :]

                            for a in range(nquads // 4):
                                qk4 = ps2.tile(
                                    [128, 4, 128], f32, tag="qk4",
                                    bufs=3 if "tpsmerge" in opts else None)
                                for qi in range(4):
                                    n0 = 32 * a + 8 * qi
                                    q_v = qkv_slice("q", n0).rearrange(
                                        "d g h -> d (g h)")
                                    k_v = qkv_slice("k", n0).rearrange(
                                        "d g h -> d (g h)")
                                    nc.tensor.matmul(
                                        qk4[:, qi, :], lhsT=q_v, rhs=k_v,
                                        start=(qi == 0), stop=False,
                                        skip_group_check=True)
                                    if "maskw" not in opts:
                                        nc.tensor.matmul(
                                            qk4[:, qi, :], lhsT=ml_sb,
                                            rhs=mr_sb,
                                            start=False, stop=(qi == 3),
                                            skip_group_check=True)
                                if "maskw" in opts:
                                    nc.tensor.matmul(
                                        qk4.rearrange("p q n -> p (q n)"),
                                        lhsT=ml_sb, rhs=mr4_sb,
                                        start=False, stop=True,
                                        skip_group_check=True)
                                att4 = attp.tile([128, 4, 128], bf16, tag="att4")
                                rs4 = attp.tile([128, 4], f32, tag="rs4")
                                if "eacc" in opts:
                                    for qi in range(4):
                                        nc.scalar.activation(
                                            att4[:, qi, :], qk4[:, qi, :],
                                            Exp, bias=ebias,
                                            accum_out=rs4[:, qi:qi + 1])
                                else:
                                    nc.scalar.activation(
                                        att4, qk4, Exp, bias=ebias)
                                    nc.vector.reduce_sum(
                                        out=rs4, in_=att4,
                                        axis=mybir.AxisListType.X)
                                rc4 = attp.tile([128, 4], f32, tag="rc4")
                                nc.vector.reciprocal(rc4, rs4)
                                for qi in range(4):
                                    if "mulact" in opts:
                                        nc.scalar.mul(
                                            att4[:, qi, :], att4[:, qi, :],
                                            rc4[:, qi:qi + 1])
                                    else:
                                        nc.vector.tensor_scalar_mul(
                                            att4[:, qi, :], att4[:, qi, :],
                                            rc4[:, qi:qi + 1])

                                if "tpsmerge" in opts:
                                    tps_ps = ps2.tile(
                                        [128, 8, 128], bf16, tag="tps",
                                        bufs=2)
                                    for qi in range(4):
                                        n0 = 32 * a + 8 * qi
                                        v_v = qkv_sb["v"][
                                            :, n0:n0 + 8, :].rearrange(
                                            "d g t -> d (g t)")
                                        nc.tensor.matmul(
                                            tps_ps[:, 4 + qi, :], lhsT=v_v,
                                            rhs=ident, is_transpose=True,
                                            start=(qi == 0), stop=False,
                                            skip_group_check=True)
                                    for qi in range(4):
                                        nc.tensor.matmul(
                                            tps_ps[:, qi, :],
                                            lhsT=att4[:, qi, :], rhs=ident,
                                            is_transpose=True,
                                            start=False, stop=(qi == 3),
                                            skip_group_check=True)
                                    tps = attp.tile(
                                        [128, 8, 128], bf16, tag="tpss")
                                    if "attcopyact" in opts:
                                        nc.scalar.copy(tps, tps_ps)
                                    else:
                                        nc.vector.tensor_copy(tps, tps_ps)
                                    attT4 = tps[:, 0:4, :]
                                    vp4 = tps[:, 4:8, :]
                                else:
                                    attT4_ps = ps2.tile(
                                        [128, 4, 128], bf16, tag="attT4")
                                    vp4_ps = ps2.tile(
                                        [128, 4, 128], bf16, tag="vp4")
                                    for qi in range(4):
                                        n0 = 32 * a + 8 * qi
                                        nc.tensor.matmul(
                                            attT4_ps[:, qi, :],
                                            lhsT=att4[:, qi, :], rhs=ident,
                                            is_transpose=True,
                                            start=(qi == 0), stop=(qi == 3),
                                            skip_group_check=True)
                                        v_v = qkv_slice("v", n0).rearrange(
                                            "d g t -> d (g t)")
                                        nc.tensor.matmul(
                                            vp4_ps[:, qi, :], lhsT=v_v,
                                            rhs=ident, is_transpose=True,
                                            start=(qi == 0), stop=(qi == 3),
                                            skip_group_check=True)
                                    attT4 = attp.tile(
                                        [128, 4, 128], bf16, tag="attT4s")
                                    if "attcopyact" in opts:
                                        nc.scalar.copy(attT4, attT4_ps)
                                    else:
                                        nc.vector.tensor_copy(attT4, attT4_ps)
                                    vp4 = attp.tile(
                                        [128, 4, 128], bf16, tag="vp4s")
                                    if "vpcopyact" in opts:
                                        nc.scalar.copy(vp4, vp4_ps)
                                    else:
                                        nc.vector.tensor_copy(vp4, vp4_ps)

                                o4_ps = ps2.tile(
                                    [128, 4, 128], f32, tag="o4",
                                    bufs=3 if "tpsmerge" in opts else None)
                                for qi in range(4):
                                    nc.tensor.matmul(
                                        o4_ps[:, qi, :],
                                        lhsT=vp4[:, qi, :],
                                        rhs=attT4[:, qi, :],
                                        start=(qi == 0), stop=(qi == 3),
                                        skip_group_check=True)
                                tc0 = 2 * a
                                half, tcl0 = tc0 // tw2, tc0 % tw2
                                dst = scr5[half][:, :, tcl0:tcl0 + 2, :].rearrange(
                                    "p (jb g) t h -> p t jb g h", jb=2)
                                nc.vector.tensor_copy(
                                    dst,
                                    o4_ps.rearrange(
                                        "p (tb jb) (g h) -> p tb jb g h",
                                        jb=2, h=16))
                            nquads_left = 0
                        else:
                            nquads_left = nquads
                        for g0 in range(
                                0 if "noatt" not in opts and nquads_left else 10**9,
                                nquads_left):
                            n0 = 8 * g0
                            j0, tc_ = n0 % 16, g0 // 2
                            q_v = qkv_sb["q"][:, n0:n0 + 8, :].rearrange(
                                "d g h -> d (g h)")
                            k_v = qkv_sb["k"][:, n0:n0 + 8, :].rearrange(
                                "d g h -> d (g h)")
                            qk_ps = ps2.tile([128, 128], f32, tag="qk")
                            att = attp.tile([128, 128], bf16, tag="att")
                            rs = attp.tile([128, 1], f32, tag="rs")
                            if "fastmask" in opts:
                                nc.tensor.matmul(
                                    qk_ps, lhsT=q_v, rhs=k_v,
                                    start=True, stop=False)
                                nc.tensor.matmul(
                                    qk_ps, lhsT=ml_sb, rhs=mr_sb,
                                    start=False, stop=True)
                                nc.scalar.activation(
                                    att, qk_ps, Exp, bias=ebias, accum_out=rs)
                            else:
                                nc.tensor.matmul(
                                    qk_ps, lhsT=q_v, rhs=k_v,
                                    start=True, stop=True)
                                if "noatt_dve" not in opts:
                                    nc.vector.tensor_add(qk_ps, qk_ps, mask_sb)
                                nc.scalar.activation(att, qk_ps, Exp, accum_out=rs)
                            if "noatt_dve" not in opts:
                                if "divnorm" in opts:
                                    nc.vector.tensor_scalar(
                                        att, att, rs, None,
                                        op0=mybir.AluOpType.divide)
                                else:
                                    rc = attp.tile([128, 1], f32, tag="rc")
                                    nc.vector.reciprocal(rc, rs)
                                    nc.vector.tensor_scalar_mul(att, att, rc)

                            attT_ps = ps2.tile([128, 128], bf16, tag="attT")
                            nc.tensor.transpose(attT_ps, att, ident)
                            attT = attp.tile([128, 128], bf16, tag="attTs")
                            nc.vector.tensor_copy(attT, attT_ps)

                            if vphoist:
                                vp = v2_sb[:, g0, :]
                            else:
                                v_v = qkv_sb["v"][:, n0:n0 + 8, :].rearrange(
                                    "d g t -> d (g t)")
                                vp_ps = ps2.tile([128, 128], bf16, tag="vp")
                                nc.tensor.transpose(vp_ps, v_v, ident)
                                vp = attp.tile([128, 128], bf16, tag="vps")
                                nc.vector.tensor_copy(vp, vp_ps)

                            o_ps = ps2.tile(
                                [128, 128], f32, tag="o",
                                bufs=2 if vphoist else None)
                            nc.tensor.matmul(
                                o_ps, lhsT=vp, rhs=attT, start=True, stop=True)
                            if "scrsplit" in opts:
                                tw2 = TW // 2
                                half, tcl = tc_ // tw2, tc_ % tw2
                                dst = scr5[half][:, j0:j0 + 8, tcl:tcl + 1, :]
                            else:
                                dst = scr4[:, j0:j0 + 8, :, tc_:tc_ + 1]
                            nc.vector.tensor_copy(
                                dst, o_ps.rearrange("p (g h) -> p g h", g=8))

            # ---------------- phase 3: output projection ----------------
            wo_v = w_d["o"].ap().rearrange("(jt jp) r -> jp jt r", jp=128)
            out_v = out_d.ap().rearrange("(rt rp) n -> rp rt n", rp=128)
            if "scrsplit" in opts:
                ch3 = npos // 2
                chunks = [(scrA, slice(0, ch3)), (scrB, slice(ch3, npos))]
            else:
                ch3 = CH
                chunks = [
                    (scr_sb, slice(c * CH, (c + 1) * CH)) for c in range(NCH)]
            if fuse:
                _c3 = tc.tile_pool(name="ps3", bufs=2, space="PSUM")
                ps3_h = _c3.__enter__()
                held.append(_c3)
                ps3_ctx = contextlib.nullcontext(ps3_h)
            else:
                ps3_ctx = tc.tile_pool(name="ps3", bufs=3, space="PSUM")
            with ps3_ctx as ps3:
                for rg in range(8):
                    w_sb = wpool.tile([128, 16, 256], bf16, tag="w")
                    nc.sync.dma_start(w_sb, wo_v[:, :, rg * 256:(rg + 1) * 256])
                    for ro in range(2):
                        r = rg * 2 + ro
                        for src, csl in chunks:
                            ps = ps3.tile([128, ch3], f32)
                            for j in range(16):
                                rhs = (src[:, j, :] if "scrsplit" in opts
                                       else src[:, j, csl])
                                nc.tensor.matmul(
                                    ps,
                                    lhsT=w_sb[:, j, ro * 128:(ro + 1) * 128],
                                    rhs=rhs,
                                    start=(j == 0), stop=(j == 15))
                            o_sb = outp.tile([128, ch3], f32)
                            nc.scalar.activation(
                                o_sb, ps, Ident, bias=b_sb["o"][:, r:r + 1])
                            nc.sync.dma_start(out_v[:, r, csl], o_sb)

            for cm in reversed(held):
                cm.__exit__(None, None, None)

    def body_v2(tc):
        """Dependency-restructured pipeline:

        q-proj, k-proj (m-outer, weights streamed once) -> attention part1
        (qk+mask matmul, exp, normalize -> att_all in SBUF; needs only q,k)
        emitted BEFORE v-proj so part1's ACT/DVE work overlaps v's dense PE
        matmuls -> part2 (transposes + att@v + scatter) -> o-projection in
        two passes (scrA, scrB; Wo streamed twice) so the list scheduler
        fills part2 latency bubbles with o-proj matmuls.
        """
        with (
            tc.tile_pool(name="consts", bufs=1) as consts,
            tc.tile_pool(name="wpool", bufs=2) as wpool,
            tc.tile_pool(name="vpool", bufs=1) as vpool,
            tc.tile_pool(name="attall", bufs=1) as attall,
            tc.tile_pool(name="attp", bufs=3) as attp,
            tc.tile_pool(name="tmp", bufs=2) as tmp,
            tc.tile_pool(name="outp", bufs=2) as outp,
        ):
            cos_sb = consts.tile([128, npos], f32)
            nc.sync.dma_start(cos_sb, cos_d.ap())
            sin_sb = consts.tile([128, npos], f32)
            nc.sync.dma_start(sin_sb, sin_d.ap())
            if "dvemask" in opts:
                mask4 = consts.tile([128, 4, 128], f32)
                for qi in range(4):
                    nc.sync.dma_start(mask4[:, qi, :], mask_d.ap())
            else:
                ml_sb = consts.tile([8, 128], bf16)
                nc.sync.dma_start(ml_sb, maskl_d.ap())
                mr4_sb = consts.tile([8, 512], bf16)
                nc.sync.dma_start(mr4_sb, maskr4_d.ap())
                ebias = consts.tile([128, 1], f32)
                nc.vector.memset(ebias, -MASKVAL)
            ident = consts.tile([128, 128], bf16)
            make_identity(nc, ident)
            b_sb = {}
            for p in ("q", "k", "v", "o"):
                b_sb[p] = consts.tile([128, 16], f32, tag=f"b_{p}", name=f"b_{p}")
                nc.sync.dma_start(b_sb[p], b_d[p].ap().rearrange("(t p) -> p t", p=128))

            att_all = attall.tile([128, NQ // 4, 4, 128], bf16)
            v_sb = vpool.tile([128, npos, 16], bf16, tag="qkv_v", name="qkv_v")

            with (
                tc.tile_pool(name="qkp", bufs=1) as qkp,
                tc.tile_pool(name="xp", bufs=1) as xp,
                tc.tile_pool(
                    name="ps1", bufs=4 if "wreuse" in opts else 3,
                    space="PSUM") as ps1,
                tc.tile_pool(name="psqk", bufs=3, space="PSUM") as psqk,
            ):
                xt_sb = xp.tile([128, 16, npos], bf16)
                xt_v = xt_d.ap().rearrange("(kt kp) n -> kp kt n", kp=128)
                nxc = max(NCH, 4)
                xcw = npos // nxc
                for xc in range(nxc):
                    csl = slice(xc * xcw, (xc + 1) * xcw)
                    nc.sync.dma_start(xt_sb[:, :, csl], xt_v[:, :, csl])

                qk_sb = {
                    p: qkp.tile([128, npos, 16], bf16,
                                tag=f"qkv_{p}", name=f"qkv_{p}")
                    for p in ("q", "k")
                }

                wreuse = "wreuse" in opts

                nobias = "nobias" in opts

                def evict(p, dst, ps, m, csl):
                    qdst = dst[:, csl, m]
                    if p == "v" or "norope" in opts:
                        if nobias:
                            nc.scalar.copy(qdst, ps)
                        else:
                            nc.scalar.activation(
                                qdst, ps, Ident, bias=b_sb[p][:, m:m + 1])
                    else:
                        if not nobias:
                            nc.vector.tensor_scalar_add(
                                ps, ps, b_sb[p][:, m:m + 1])
                        t1 = tmp.tile([128, CH], f32, tag="t1")
                        nc.vector.tensor_mul(t1, ps, cos_sb[:, csl])
                        tsw = tmp.tile([128, CH], f32, tag="tsw")
                        nc.scalar.copy(tsw[0:64, :], ps[64:128, :])
                        nc.scalar.copy(tsw[64:128, :], ps[0:64, :])
                        nc.vector.tensor_mul(tsw, tsw, sin_sb[:, csl])
                        nc.vector.tensor_add(qdst, t1, tsw)

                wlay = "wlay" in opts

                def proj(p, dst):
                    wv_d = w_d[p].ap().rearrange("(kt kp) e -> kp kt e", kp=128)
                    for mg in range(8):
                        w_sb = wpool.tile([128, 16, 256], bf16, tag="w")
                        if wlay:
                            nc.sync.dma_start(w_sb, wr_d[p].ap()[mg])
                        else:
                            nc.sync.dma_start(
                                w_sb, wv_d[:, :, mg * 256:(mg + 1) * 256])
                        for mo in range(2):
                            m = mg * 2 + mo
                            if wreuse:
                                # kt-outer: consecutive matmuls share the
                                # stationary operand (one weight load per
                                # kt feeds both column chunks)
                                pss = [
                                    ps1.tile([128, CH], f32, tag="ps1",
                                             name=f"ps1_{ch}")
                                    for ch in range(NCH)
                                ]
                                for kt in range(16):
                                    for ch in range(NCH):
                                        nc.tensor.matmul(
                                            pss[ch],
                                            lhsT=w_sb[:, kt, mo * 128:(mo + 1) * 128],
                                            rhs=xt_sb[:, kt, ch * CH:(ch + 1) * CH],
                                            start=(kt == 0), stop=(kt == 15))
                                for ch in range(NCH):
                                    evict(p, dst, pss[ch], m,
                                          slice(ch * CH, (ch + 1) * CH))
                            else:
                                for ch in range(NCH):
                                    csl = slice(ch * CH, (ch + 1) * CH)
                                    ps = ps1.tile([128, CH], f32, tag="ps1")
                                    for kt in range(16):
                                        nc.tensor.matmul(
                                            ps,
                                            lhsT=w_sb[:, kt, mo * 128:(mo + 1) * 128],
                                            rhs=xt_sb[:, kt, csl],
                                            start=(kt == 0), stop=(kt == 15))
                                    evict(p, dst, ps, m, csl)

                proj("q", qk_sb["q"])
                proj("k", qk_sb["k"])

                # attention part1: qk + mask -> exp -> normalize -> att_all
                for a in range(0 if "projonly" in opts else NQ // 4):
                    qk4 = psqk.tile([128, 4, 128], f32, tag="qk4")
                    for qi in range(4):
                        n0 = 32 * a + 8 * qi
                        q_v = qk_sb["q"][:, n0:n0 + 8, :].rearrange(
                            "d g h -> d (g h)")
                        k_v = qk_sb["k"][:, n0:n0 + 8, :].rearrange(
                            "d g h -> d (g h)")
                        nc.tensor.matmul(
                            qk4[:, qi, :], lhsT=q_v, rhs=k_v,
                            start=(qi == 0),
                            stop=("dvemask" in opts and qi == 3),
                            skip_group_check=True)
                    att4 = att_all[:, a, :, :]
                    if "dvemask" in opts:
                        nc.vector.tensor_add(qk4, qk4, mask4)
                        nc.scalar.activation(att4, qk4, Exp)
                    else:
                        nc.tensor.matmul(
                            qk4.rearrange("p q n -> p (q n)"),
                            lhsT=ml_sb, rhs=mr4_sb,
                            start=False, stop=True, skip_group_check=True)
                        nc.scalar.activation(att4, qk4, Exp, bias=ebias)
                    rs4 = attp.tile([128, 4], f32, tag="rs4")
                    nc.vector.reduce_sum(
                        out=rs4, in_=att4, axis=mybir.AxisListType.X)
                    rc4 = attp.tile([128, 4], f32, tag="rc4")
                    nc.vector.reciprocal(rc4, rs4)
                    for qi in range(4):
                        nc.vector.tensor_scalar_mul(
                            att4[:, qi, :], att4[:, qi, :], rc4[:, qi:qi + 1])

                proj("v", v_sb)

            # ---- part2 + o-projection, interleaved by the scheduler ----
            with (
                tc.tile_pool(name="scrp", bufs=1) as scrp,
                tc.tile_pool(name="ps2", bufs=2, space="PSUM") as ps2,
                tc.tile_pool(name="ps3", bufs=4, space="PSUM") as ps3,
            ):
                tw2 = TW // 2
                scrA = scrp.tile([128, 16, npos // 2], bf16, tag="scrA")
                scrB = scrp.tile([128, 16, npos // 2], bf16, tag="scrB")
                scr5 = [
                    s.rearrange("p j (t h) -> p j t h", h=16)
                    for s in (scrA, scrB)
                ]

                def part2(a):
                    tps_ps = ps2.tile([128, 8, 128], bf16, tag="tps")
                    for qi in range(4):
                        n0 = 32 * a + 8 * qi
                        v_v = v_sb[:, n0:n0 + 8, :].rearrange(
                            "d g t -> d (g t)")
                        nc.tensor.matmul(
                            tps_ps[:, 4 + qi, :], lhsT=v_v,
                            rhs=ident, is_transpose=True,
                            start=(qi == 0), stop=False,
                            skip_group_check=True)
                    for qi in range(4):
                        nc.tensor.matmul(
                            tps_ps[:, qi, :],
                            lhsT=att_all[:, a, qi, :], rhs=ident,
                            is_transpose=True,
                            start=False, stop=(qi == 3),
                            skip_group_check=True)
                    tps = attp.tile([128, 8, 128], bf16, tag="tpss")
                    nc.scalar.copy(tps, tps_ps)
                    o4_ps = ps2.tile([128, 4, 128], f32, tag="o4")
                    for qi in range(4):
                        nc.tensor.matmul(
                            o4_ps[:, qi, :],
                            lhsT=tps[:, 4 + qi, :],
                            rhs=tps[:, qi, :],
                            start=(qi == 0), stop=(qi == 3),
                            skip_group_check=True)
                    tc0 = 2 * a
                    half, tcl0 = tc0 // tw2, tc0 % tw2
                    dst = scr5[half][:, :, tcl0:tcl0 + 2, :].rearrange(
                        "p (jb g) t h -> p t jb g h", jb=2)
                    nc.vector.tensor_copy(
                        dst,
                        o4_ps.rearrange(
                            "p (tb jb) (g h) -> p tb jb g h",
                            jb=2, h=16))

                wo_v = w_d["o"].ap().rearrange("(jt jp) r -> jp jt r", jp=128)
                out_v = out_d.ap().rearrange("(rt rp) n -> rp rt n", rp=128)

                def oproj_pass(src, osl):
                    for rg in range(8):
                        w_sb = wpool.tile([128, 16, 256], bf16, tag="w")
                        if "wlay" in opts:
                            nc.sync.dma_start(w_sb, wr_d["o"].ap()[rg])
                        else:
                            nc.sync.dma_start(
                                w_sb, wo_v[:, :, rg * 256:(rg + 1) * 256])
                        for ro in range(2):
                            r = rg * 2 + ro
                            ps = ps3.tile([128, npos // 2], f32, tag="ps3")
                            for j in range(16):
                                nc.tensor.matmul(
                                    ps,
                                    lhsT=w_sb[:, j, ro * 128:(ro + 1) * 128],
                                    rhs=src[:, j, :],
                                    start=(j == 0), stop=(j == 15))
                            o_sb = outp.tile([128, npos // 2], out_dt)
                            if "nobias" in opts:
                                nc.scalar.copy(o_sb, ps)
                            else:
                                nc.scalar.activation(
                                    o_sb, ps, Ident, bias=b_sb["o"][:, r:r + 1])
                            nc.sync.dma_start(out_v[:, r, osl], o_sb)

                if "projonly" in opts:
                    nc.vector.memset(scrA, 0.0)
                    nc.vector.memset(scrB, 0.0)
                    oproj_pass(scrA, slice(0, npos // 2))
                    oproj_pass(scrB, slice(npos // 2, npos))
                else:
                    for a in range(NQ // 8):
                        part2(a)
                    oproj_pass(scrA, slice(0, npos // 2))
                    for a in range(NQ // 8, NQ // 4):
                        part2(a)
                    oproj_pass(scrB, slice(npos // 2, npos))

    with tile.TileContext(nc) as tc:
        for _ in range(reps):
            if "v2" in opts:
                body_v2(tc)
            else:
                body(tc)

    nc.compile()
    _NC_CACHE[key] = nc
    return nc


# ---------------------------------------------------------------------------
# Runner (PJRT via axon, cached jitted callable)
# ---------------------------------------------------------------------------

_RUNNER_CACHE = {}


def make_runner(nc, ncores=NCORES):
    """Returns run(in_maps) -> list of per-core output dicts.

    Mirrors bass2jax.run_bass_via_pjrt but caches the jitted callable and
    does NOT donate output buffers (kernel writes every output element), so
    repeated timed calls don't re-trace or re-transfer.
    """
    key = id(nc)
    if key in _RUNNER_CACHE:
        return _RUNNER_CACHE[key]

    import jax
    import numpy as _np
    from jax.sharding import Mesh, PartitionSpec
    from jax.experimental.shard_map import shard_map
    from concourse import mybir
    from concourse import bass2jax
    from concourse.bass2jax import (
        _bass_exec_p, install_neuronx_cc_hook, partition_id_tensor)

    install_neuronx_cc_hook()

    partition_name = (
        nc.partition_id_tensor.name if nc.partition_id_tensor else None)
    in_names, out_names, out_avals, zero_outs = [], [], [], []
    for alloc in nc.m.functions[0].allocations:
        if not isinstance(alloc, mybir.MemoryLocationSet):
            continue
        name = alloc.memorylocations[0].name
        if alloc.kind == "ExternalInput":
            if name == partition_name:
                continue
            in_names.append(name)
        elif alloc.kind == "ExternalOutput":
            shape = tuple(alloc.tensor_shape)
            dtype = mybir.dt.np(alloc.dtype)
            out_names.append(name)
            out_avals.append(jax.core.ShapedArray(shape, dtype))
            zero_outs.append(_np.zeros(shape, dtype))
    n_params = len(in_names)
    all_in_names = in_names + out_names
    if partition_name is not None:
        all_in_names = all_in_names + [partition_name]

    def _body(*args):
        operands = list(args)
        if partition_name is not None:
            operands.append(partition_id_tensor())
        outs = _bass_exec_p.bind(
            *operands,
            out_avals=tuple(out_avals),
            in_names=tuple(all_in_names),
            out_names=tuple(out_names),
            lowering_input_output_aliases=(),
            sim_require_finite=True,
            sim_require_nnan=True,
            nc=nc,
        )
        return tuple(outs)

    devices = jax.devices()[:ncores]
    mesh = Mesh(np.asarray(devices), ("core",))
    n_outs = len(out_names)
    jitted = jax.jit(
        shard_map(
            _body, mesh=mesh,
            in_specs=(PartitionSpec("core"),) * (n_params + n_outs),
            out_specs=(PartitionSpec("core"),) * n_outs,
            check_rep=False,
        ),
        keep_unused=True,
    )

    zeros_dev = [
        jax.device_put(
            _np.zeros((ncores * z.shape[0], *z.shape[1:]), z.dtype))
        for z in zero_outs
    ]

    def put(in_maps):
        concat = [
            _np.concatenate([_np.asarray(m[name]) for m in in_maps], axis=0)
            for name in in_names
        ]
        return [jax.device_put(a) for a in concat]

    def run_dev(in_dev):
        outs = jitted(*in_dev, *zeros_dev)
        jax.block_until_ready(outs)
        return outs

    def run(in_maps):
        outs = run_dev(put(in_maps))
        res = []
        for c in range(len(in_maps)):
            res.append({
                name: _np.asarray(outs[i]).reshape(
                    len(in_maps), *out_avals[i].shape)[c]
                for i, name in enumerate(out_names)
            })
        return res

    run.put = put
    run.run_dev = run_dev
    run.out_names = out_names
    _RUNNER_CACHE[key] = run
    return run


DEFAULT_OPTS = frozenset({"v2"})


def kernel(**inputs) -> np.ndarray:
    inputs = {k: np.asarray(v) for k, v in inputs.items()}
    opts = DEFAULT_OPTS
    in_maps, meta = prepare_host(**inputs)
    nc = build_nc(NPOS, opts=opts)
    run = make_runner(nc, NCORES)
    outs = run(in_maps)
    layout = "t_h" if ("scrsplit" in opts or "v2" in opts) else "h_t"
    return assemble_output(outs, meta, NPOS, layout)



# revision 32
# speedup vs baseline: 3.0046x; 1.0047x over previous
"""Trainium2 Bass kernel for nn_MultiHeadAttention_81664508166458.

Reference computes a "cross-head" MHA: per (batch, position) the attention
matrix is HxH (H=16 heads), contracting head_dim D=128. Every position is
independent, so we shard the 8192 (batch, position) pairs across 8 cores
(1024 each), fully data-parallel, no collectives.

Host-side preprocessing (part of sharding, not timed device work):
  - weights transposed to [e_in, e_out] (k-major) and cast to bf16
  - RoPE pair permutation baked into Wq/Wk rows: head-local dim d' with
    x0 (even d) in d'=[0,64) and x1 (odd d) in d'=[64,128) so the rotation
    becomes same-partition table multiplies plus a half-swap
  - 1/sqrt(D) attention scale baked into Wq/bq
  - x transposed to [e_in, n] bf16
  - cos/sin tables and the block-diagonal softmax mask precomputed

Device pipeline per core (all matmuls bf16 with fp32 PSUM accumulation;
"v2" dependency-restructured flow, chosen so the Tile list scheduler keeps
the PE near-saturated — PE work is ~1.13M cycles and is the binding
resource):
  1. q-proj, k-proj [d, n, h] = W*T.T @ xT (16 e-tiles x 16 k-tiles,
     N=512), RoPE fused into the PSUM eviction (DVE table mults + ACT
     half-swap)
  2. attention part1 (only needs q,k — emitted BEFORE the v projection so
     its exp/normalize ACT+DVE work overlaps v's dense PE matmuls): per
     4-quad group qk+mask matmuls -> exp -> row-sum -> normalize, with
     normalized att kept in a persistent 4MB SBUF tile
  3. v-proj (dense PE, runs under part1's ACT/DVE tail)
  4. part2 per group: PE-transpose att and v slices, O^T = vp @ attT,
     DVE-scatter into scr; emitted as [A-half, o-proj pass A, B-half,
     o-proj pass B] so o-proj matmuls fill part2's latency bubbles
  5. outT[r, (t,h)] = WoT.T @ scr (+bo) in two passes (Wo streamed twice),
     DMA to DRAM [E, n] (host transposes)
"""

import numpy as np
import ml_dtypes

B, S, E = 4, 2048, 2048
H, D = 16, 128
NCORES = 8
CORES_PER_BATCH = NCORES // B          # 2
NPOS = S // CORES_PER_BATCH            # 1024 positions per core
THETA = 10000.0
MASK_NEG = -30000.0

BF16 = ml_dtypes.bfloat16

# ---------------------------------------------------------------------------
# Host-side preprocessing
# ---------------------------------------------------------------------------


def _rope_perm():
    """P_IDX[new] = old row index: x0 (even d) -> d'=[0,64), x1 (odd) -> [64,128)."""
    p = np.empty(E, np.int64)
    for h in range(H):
        base = h * D
        i = np.arange(D // 2)
        p[base + i] = base + 2 * i
        p[base + 64 + i] = base + 2 * i + 1
    return p


def _rope_tables(npos, offset):
    """cos table C[p, n] and signed sin table S[p, n], p in [0,128)."""
    inv = 1.0 / (THETA ** (np.arange(0, D, 2, dtype=np.float64) / D))  # [64]
    pos = np.arange(offset, offset + npos, dtype=np.float64)
    fr = np.outer(inv, pos)  # [64, npos]
    c = np.cos(fr).astype(np.float32)
    s = np.sin(fr).astype(np.float32)
    cos_b = np.concatenate([c, c], axis=0)            # [128, npos]
    sin_b = np.concatenate([-s, s], axis=0)           # signed
    return np.ascontiguousarray(cos_b), np.ascontiguousarray(sin_b)


def _blockdiag_mask():
    m = np.full((128, 128), MASK_NEG, np.float32)
    for g in range(8):
        m[g * 16:(g + 1) * 16, g * 16:(g + 1) * 16] = 0.0
    return m


# exact bf16-representable mask magnitude (softmax is shift-invariant, but we
# keep the on-diagonal shift exactly zero: +MASKVAL via matmul, -MASKVAL bias)
MASKVAL = float(np.float32(BF16(30000.0)))


def _mask_mm():
    """K=8 rank-8 matmul operands adding +MASKVAL on the block diagonal.
    maskl[g, p] = MASKVAL if p//16==g else 0 ; maskr[g, f] = 1 if f//16==g."""
    ind = np.zeros((8, 128), np.float32)
    for g in range(8):
        ind[g, g * 16:(g + 1) * 16] = 1.0
    return (ind * MASKVAL).astype(BF16), ind.astype(BF16)


def _repack_w(wt):
    """[E_in=(kt kp), E_out=(mg e)] -> [mg, kp, kt, e] with e=256."""
    w4 = np.asarray(wt).reshape(16, 128, 8, 256)      # kt kp mg e
    return np.ascontiguousarray(np.transpose(w4, (2, 1, 0, 3)))


def prepare_host(x, Wq, bq, Wk, bk, Wv, bv, Wo, bo, npos=NPOS, ncores=NCORES):
    """Returns (shared weight arrays dict, list of per-core in_maps)."""
    x = np.asarray(x, np.float32)
    perm = _rope_perm()
    scale = np.float32(1.0 / np.sqrt(D))

    wqt = np.ascontiguousarray((np.asarray(Wq, np.float32)[perm, :] * scale).T).astype(BF16)
    wkt = np.ascontiguousarray(np.asarray(Wk, np.float32)[perm, :].T).astype(BF16)
    wvt = np.ascontiguousarray(np.asarray(Wv, np.float32).T).astype(BF16)
    wot = np.ascontiguousarray(np.asarray(Wo, np.float32).T).astype(BF16)
    bq_p = (np.asarray(bq, np.float32)[perm] * scale).copy()
    bk_p = np.asarray(bk, np.float32)[perm].copy()
    bv_p = np.asarray(bv, np.float32).copy()
    bo_p = np.asarray(bo, np.float32).copy()
    mask = _blockdiag_mask()
    maskl, maskr = _mask_mm()

    in_maps = []
    meta = []
    for c in range(ncores):
        bc = c // CORES_PER_BATCH
        o = (c % CORES_PER_BATCH) * npos
        xc = x[bc, o:o + npos, :]                      # [npos, E]
        xt = np.ascontiguousarray(xc.T).astype(BF16)   # [E, npos]
        cos_b, sin_b = _rope_tables(npos, o)
        in_maps.append({
            "xt": xt, "wqt": wqt, "wkt": wkt, "wvt": wvt, "wot": wot,
            "bq": bq_p, "bk": bk_p, "bv": bv_p, "bo": bo_p,
            "cosb": cos_b, "sinb": sin_b, "mask": mask,
            "maskl": maskl, "maskr": maskr,
            "maskr4": np.ascontiguousarray(np.tile(maskr, (1, 4))),
            # weights repacked [mg, kp, kt, e256] so each per-partition DMA
            # row is 16*256*2 = 8KB contiguous (full DMA line efficiency)
            "wqtr": _repack_w(wqt), "wktr": _repack_w(wkt),
            "wvtr": _repack_w(wvt), "wotr": _repack_w(wot),
        })
        meta.append((bc, o))
    return in_maps, meta


def assemble_output(outs, meta, npos=NPOS, layout="h_t"):
    """outs: list of per-core {'outt': [E, npos] f32}. Returns [B, S, E].

    layout "h_t": outt col = h*tw + tc (tc local).
    layout "t_h": outt col = tc*16 + h (scrsplit build).
    """
    full = np.empty((B, S, E), np.float32)
    tw = npos // 16
    for (bc, o), res in zip(meta, outs):
        outt = res["outt"]
        if layout == "h_t":
            v = outt.reshape(E, H, tw)           # [E, h, tc]
            v = np.transpose(v, (1, 2, 0))       # [h, tc, E]
        else:
            v = outt.reshape(E, tw, H)           # [E, tc, h]
            v = np.transpose(v, (2, 1, 0))       # [h, tc, E]
        t0 = o // 16
        for h in range(H):
            full[bc, h * 128 + t0: h * 128 + t0 + tw, :] = v[h]
    return full


# ---------------------------------------------------------------------------
# Numpy emulator of the exact device dataflow (index-math validation)
# ---------------------------------------------------------------------------


def emulate_core(im, npos=NPOS, layout="h_t"):
    f32 = np.float32
    xt = im["xt"].astype(f32)
    qT = (im["wqt"].astype(f32).T @ xt) + im["bq"][:, None]   # [E, n]
    kT = (im["wkt"].astype(f32).T @ xt) + im["bk"][:, None]
    vT = (im["wvt"].astype(f32).T @ xt) + im["bv"][:, None]
    C, Sg = im["cosb"].astype(f32), im["sinb"].astype(f32)

    def rope(t):
        t3 = t.reshape(H, D, npos)                            # [h, d', n]
        sw = np.concatenate([t3[:, 64:, :], t3[:, :64, :]], axis=1)
        r = t3 * C[None] + sw * Sg[None]
        return r.astype(BF16).astype(f32)

    qr, kr = rope(qT), rope(kT)
    vb = vT.astype(BF16).astype(f32).reshape(H, D, npos)
    scr = np.zeros((D, 16, npos), f32)                        # [d, j, h*tw+tc]
    tw = npos // 16
    for g0 in range(npos // 8):
        n0 = 8 * g0
        j0, tc = n0 % 16, g0 // 2
        q_blk = qr[:, :, n0:n0 + 8]                           # [h, d, g]
        k_blk = kr[:, :, n0:n0 + 8]
        lhs = np.transpose(q_blk, (1, 2, 0)).reshape(D, 128)  # [d, (g,h)]
        rhs = np.transpose(k_blk, (1, 2, 0)).reshape(D, 128)  # [d, (g,t)]
        qk = lhs.T @ rhs + im["mask"]
        e = np.exp(qk)
        att = (e / e.sum(1, keepdims=True)).astype(BF16).astype(f32)
        vm = np.transpose(vb[:, :, n0:n0 + 8], (1, 2, 0)).reshape(D, 128)  # [d,(g,t)]
        # out2T[d, (g,h)] = sum_{(g,t)} vm[d, (g,t)] * att[(g,h), (g,t)]
        o2 = vm @ att.T                 # [d, (g,h)]
        o2v = o2.reshape(D, 8, 16)
        if layout == "h_t":
            scr.reshape(D, 16, H, tw)[:, j0:j0 + 8, :, tc] = o2v
        else:
            scr.reshape(D, 16, tw, H)[:, j0:j0 + 8, tc, :] = o2v
    # scr[d, j, col] -> rhs row e=(j*128+d)
    rhs_full = np.transpose(scr, (1, 0, 2)).reshape(16 * D, npos).astype(BF16).astype(f32)
    outt = im["wot"].astype(f32).T @ rhs_full + im["bo"][:, None]
    return {"outt": outt.astype(f32)}


def emulate_full(inputs, npos=NPOS, ncores=NCORES, layout="h_t"):
    in_maps, meta = prepare_host(**inputs, npos=npos, ncores=ncores)
    outs = [emulate_core(im, npos, layout) for im in in_maps]
    return assemble_output(outs, meta, npos, layout)


# ---------------------------------------------------------------------------
# Bass kernel
# ---------------------------------------------------------------------------

_NC_CACHE = {}


def build_nc(npos=NPOS, reps=1, opts=frozenset()):
    import concourse.bass as bass
    import concourse.tile as tile
    from concourse import bacc, mybir
    from concourse.masks import make_identity

    opts = frozenset(opts)
    key = (npos, reps, opts)
    if key in _NC_CACHE:
        return _NC_CACHE[key]

    f32, bf16 = mybir.dt.float32, mybir.dt.bfloat16
    CH = 256 if "ch256" in opts else min(512, npos)  # free-dim chunk
    NCH = npos // CH
    TW = npos // 16
    NQ = npos // 8               # number of 8-position quads

    nc = bacc.Bacc("TRN2", target_bir_lowering=False, debug=False)

    xt_d = nc.dram_tensor("xt", [E, npos], bf16, kind="ExternalInput")
    w_d = {
        "q": nc.dram_tensor("wqt", [E, E], bf16, kind="ExternalInput"),
        "k": nc.dram_tensor("wkt", [E, E], bf16, kind="ExternalInput"),
        "v": nc.dram_tensor("wvt", [E, E], bf16, kind="ExternalInput"),
        "o": nc.dram_tensor("wot", [E, E], bf16, kind="ExternalInput"),
    }
    wr_d = {
        "q": nc.dram_tensor("wqtr", [8, 128, 16, 256], bf16, kind="ExternalInput"),
        "k": nc.dram_tensor("wktr", [8, 128, 16, 256], bf16, kind="ExternalInput"),
        "v": nc.dram_tensor("wvtr", [8, 128, 16, 256], bf16, kind="ExternalInput"),
        "o": nc.dram_tensor("wotr", [8, 128, 16, 256], bf16, kind="ExternalInput"),
    }
    b_d = {
        "q": nc.dram_tensor("bq", [E], f32, kind="ExternalInput"),
        "k": nc.dram_tensor("bk", [E], f32, kind="ExternalInput"),
        "v": nc.dram_tensor("bv", [E], f32, kind="ExternalInput"),
        "o": nc.dram_tensor("bo", [E], f32, kind="ExternalInput"),
    }
    cos_d = nc.dram_tensor("cosb", [128, npos], f32, kind="ExternalInput")
    sin_d = nc.dram_tensor("sinb", [128, npos], f32, kind="ExternalInput")
    mask_d = nc.dram_tensor("mask", [128, 128], f32, kind="ExternalInput")
    maskl_d = nc.dram_tensor("maskl", [8, 128], bf16, kind="ExternalInput")
    maskr_d = nc.dram_tensor("maskr", [8, 128], bf16, kind="ExternalInput")
    maskr4_d = nc.dram_tensor("maskr4", [8, 512], bf16, kind="ExternalInput")
    out_dt = bf16 if "obf16" in opts else f32
    out_d = nc.dram_tensor("outt", [E, npos], out_dt, kind="ExternalOutput")

    Exp = mybir.ActivationFunctionType.Exp
    Ident = mybir.ActivationFunctionType.Identity

    def body(tc):
        with (
            tc.tile_pool(name="consts", bufs=1) as consts,
            tc.tile_pool(name="wpool", bufs=2) as wpool,
            tc.tile_pool(name="scrp", bufs=1) as scrp,
            tc.tile_pool(
                name="tmp", bufs=3 if "tmpb3" in opts else 2) as tmp,
            tc.tile_pool(
                name="attp", bufs=4 if "attb4" in opts else 3) as attp,
            tc.tile_pool(
                name="outp", bufs=3 if "outb3" in opts else 2) as outp,
        ):
            cos_sb = consts.tile([128, npos], f32)
            nc.sync.dma_start(cos_sb, cos_d.ap())
            sin_sb = consts.tile([128, npos], f32)
            nc.sync.dma_start(sin_sb, sin_d.ap())
            if "fastmask" in opts:
                ml_sb = consts.tile([8, 128], bf16)
                nc.sync.dma_start(ml_sb, maskl_d.ap())
                mr_sb = consts.tile([8, 128], bf16)
                nc.sync.dma_start(mr_sb, maskr_d.ap())
                ebias = consts.tile([128, 1], f32)
                nc.vector.memset(ebias, -MASKVAL)
                if "maskw" in opts:
                    mr4_sb = consts.tile([8, 512], bf16)
                    nc.sync.dma_start(mr4_sb, maskr4_d.ap())
            else:
                mask_sb = consts.tile([128, 128], f32)
                nc.sync.dma_start(mask_sb, mask_d.ap())
            ident = consts.tile([128, 128], bf16)
            make_identity(nc, ident)
            b_sb = {}
            for p in ("q", "k", "v", "o"):
                b_sb[p] = consts.tile([128, 16], f32, tag=f"b_{p}", name=f"b_{p}")
                nc.sync.dma_start(b_sb[p], b_d[p].ap().rearrange("(t p) -> p t", p=128))

            if "scrsplit" in opts:
                scrA = scrp.tile([128, 16, npos // 2], bf16, tag="scrA")
                scrB = scrp.tile([128, 16, npos // 2], bf16, tag="scrB")
            else:
                scr_sb = scrp.tile([128, 16, npos], bf16)

            with tc.tile_pool(name="qkvp", bufs=1) as qkvp:
                # layout [d, n, h]: per-quad (g,h)/(g,t) views are contiguous
                if "chouter" in opts:
                    # per-chunk tiles so attention can start once a chunk's
                    # projections finish (tile-granular RAW deps)
                    qkv_ch = {
                        p: [
                            qkvp.tile([128, CH, 16], bf16,
                                      tag=f"qkv_{p}{c}", name=f"qkv_{p}{c}")
                            for c in range(NCH)
                        ]
                        for p in ("q", "k", "v")
                    }
                else:
                    qkv_sb = {
                        p: qkvp.tile([128, npos, 16], bf16,
                                     tag=f"qkv_{p}", name=f"qkv_{p}")
                        for p in ("q", "k", "v")
                    }

                # ---------------- phase 1: projections ----------------
                import contextlib
                fuse = "fuse" in opts
                held = []
                xp_ctx = tc.tile_pool(name="xp", bufs=1)
                if fuse:
                    # keep all PSUM pools open across phases (2+4+2=8 banks)
                    # so the scheduler can fill attention-chain PE stalls
                    # with projection/O-proj matmuls
                    ps1_cm = tc.tile_pool(name="ps1", bufs=2, space="PSUM")
                    ps1_h = ps1_cm.__enter__()
                    held.append(ps1_cm)
                    ps1_ctx = contextlib.nullcontext(ps1_h)
                else:
                    ps1_ctx = tc.tile_pool(
                        name="ps1", bufs=4 if "ps1b4" in opts else 3,
                        space="PSUM")
                with (xp_ctx as xp, ps1_ctx as ps1):
                    xt_sb = xp.tile([128, 16, npos], bf16)
                    nc.sync.dma_start(
                        xt_sb, xt_d.ap().rearrange("(kt kp) n -> kp kt n", kp=128))

                    if "chouter" in opts:
                        loop_iter = [
                            (p, ch, mg)
                            for p in ("q", "k", "v")
                            for ch in range(NCH)
                            for mg in range(8)
                        ]
                    else:
                        loop_iter = [
                            (p, None, mg)
                            for p in ("q", "k", "v")
                            for mg in range(8)
                        ]
                    for p, ch_o, mg in loop_iter:
                        wv_d = w_d[p].ap().rearrange("(kt kp) e -> kp kt e", kp=128)
                        if True:
                            w_sb = wpool.tile([128, 16, 256], bf16, tag="w")
                            nc.sync.dma_start(
                                w_sb, wv_d[:, :, mg * 256:(mg + 1) * 256])
                            for mo in range(2):
                                m = mg * 2 + mo
                                for ch in ([ch_o] if ch_o is not None
                                           else range(NCH)):
                                    csl = slice(ch * CH, (ch + 1) * CH)
                                    ps = ps1.tile([128, CH], f32)
                                    for kt in range(16):
                                        nc.tensor.matmul(
                                            ps,
                                            lhsT=w_sb[:, kt, mo * 128:(mo + 1) * 128],
                                            rhs=xt_sb[:, kt, csl],
                                            start=(kt == 0), stop=(kt == 15))
                                    if "chouter" in opts:
                                        qdst = qkv_ch[p][ch][:, :, m]
                                    else:
                                        qdst = qkv_sb[p][:, csl, m]
                                    if p == "v" or "norope" in opts:
                                        nc.scalar.activation(
                                            qdst, ps, Ident,
                                            bias=b_sb["v"][:, m:m + 1])
                                    else:
                                        nc.vector.tensor_scalar_add(
                                            ps, ps, b_sb[p][:, m:m + 1])
                                        t1 = tmp.tile([128, CH], f32, tag="t1")
                                        nc.vector.tensor_mul(t1, ps, cos_sb[:, csl])
                                        tsw = tmp.tile([128, CH], f32, tag="tsw")
                                        nc.scalar.copy(tsw[0:64, :], ps[64:128, :])
                                        nc.scalar.copy(tsw[64:128, :], ps[0:64, :])
                                        nc.vector.tensor_mul(tsw, tsw, sin_sb[:, csl])
                                        nc.vector.tensor_add(qdst, t1, tsw)

                # ---------------- phase 2: attention ----------------
                if "scrsplit" in opts:
                    scr5 = [
                        s.rearrange("p j (t h) -> p j t h", h=16)
                        for s in (scrA, scrB)
                    ]
                else:
                    scr4 = scr_sb.rearrange("p j (h t) -> p j h t", h=16)
                nquads = npos // 8
                vphoist = "vphoist" in opts

                with tc.tile_pool(name="v2p", bufs=1) as v2p:
                    if vphoist:
                        v2_sb = v2p.tile([128, nquads, 128], bf16)
                        with tc.tile_pool(
                                name="vpps", bufs=4, space="PSUM") as vpps:
                            for g0 in range(nquads):
                                n0 = 8 * g0
                                v_v = qkv_sb["v"][:, n0:n0 + 8, :].rearrange(
                                    "d g t -> d (g t)")
                                vp_ps = vpps.tile([128, 128], bf16, tag="vp")
                                nc.tensor.transpose(vp_ps, v_v, ident)
                                nc.vector.tensor_copy(v2_sb[:, g0, :], vp_ps)

                    if fuse:
                        _c = tc.tile_pool(name="ps2", bufs=1, space="PSUM")
                        ps2_h = _c.__enter__()
                        held.append(_c)
                        ps2_cm = contextlib.nullcontext(ps2_h)
                    elif vphoist:
                        ps2_cm = tc.tile_pool(name="ps2", bufs=3, space="PSUM")
                    else:
                        ps2_cm = tc.tile_pool(name="ps2", bufs=2, space="PSUM")
                    with ps2_cm as ps2:
                        if "noatt" in opts:
                            if "scrsplit" in opts:
                                nc.vector.memset(scrA, 0.0)
                                nc.vector.memset(scrB, 0.0)
                            else:
                                nc.vector.memset(scr_sb, 0.0)
                        if "qbatch" in opts:
                            assert {"fastmask", "scrsplit"} <= opts
                            tw2 = TW // 2

                            def qkv_slice(p, n0):
                                if "chouter" in opts:
                                    return qkv_ch[p][n0 // CH][
                                        :, n0 % CH:n0 % CH + 8, :]
                                return qkv_sb[p][:, n0:n0 + 8, :]

                            for a in range(nquads // 4):
                                qk4 = ps2.tile(
                                    [128, 4, 128], f32, tag="qk4",
                                    bufs=3 if "tpsmerge" in opts else None)
                                for qi in range(4):
                                    n0 = 32 * a + 8 * qi
                                    q_v = qkv_slice("q", n0).rearrange(
                                        "d g h -> d (g h)")
                                    k_v = qkv_slice("k", n0).rearrange(
                                        "d g h -> d (g h)")
                                    nc.tensor.matmul(
                                        qk4[:, qi, :], lhsT=q_v, rhs=k_v,
                                        start=(qi == 0), stop=False,
                                        skip_group_check=True)
                                    if "maskw" not in opts:
                                        nc.tensor.matmul(
                                            qk4[:, qi, :], lhsT=ml_sb,
                                            rhs=mr_sb,
                                            start=False, stop=(qi == 3),
                                            skip_group_check=True)
                                if "maskw" in opts:
                                    nc.tensor.matmul(
                                        qk4.rearrange("p q n -> p (q n)"),
                                        lhsT=ml_sb, rhs=mr4_sb,
                                        start=False, stop=True,
                                        skip_group_check=True)
                                att4 = attp.tile([128, 4, 128], bf16, tag="att4")
                                rs4 = attp.tile([128, 4], f32, tag="rs4")
                                if "eacc" in opts:
                                    for qi in range(4):
                                        nc.scalar.activation(
                                            att4[:, qi, :], qk4[:, qi, :],
                                            Exp, bias=ebias,
                                            accum_out=rs4[:, qi:qi + 1])
                                else:
                                    nc.scalar.activation(
                                        att4, qk4, Exp, bias=ebias)
                                    nc.vector.reduce_sum(
                                        out=rs4, in_=att4,
                                        axis=mybir.AxisListType.X)
                                rc4 = attp.tile([128, 4], f32, tag="rc4")
                                nc.vector.reciprocal(rc4, rs4)
                                for qi in range(4):
                                    if "mulact" in opts:
                                        nc.scalar.mul(
                                            att4[:, qi, :], att4[:, qi, :],
                                            rc4[:, qi:qi + 1])
                                    else:
                                        nc.vector.tensor_scalar_mul(
                                            att4[:, qi, :], att4[:, qi, :],
                                            rc4[:, qi:qi + 1])

                                if "tpsmerge" in opts:
                                    tps_ps = ps2.tile(
                                        [128, 8, 128], bf16, tag="tps",
                                        bufs=2)
                                    for qi in range(4):
                                        n0 = 32 * a + 8 * qi
                                        v_v = qkv_sb["v"][
                                            :, n0:n0 + 8, :].rearrange(
                                            "d g t -> d (g t)")
                                        nc.tensor.matmul(
                                            tps_ps[:, 4 + qi, :], lhsT=v_v,
                                            rhs=ident, is_transpose=True,
                                            start=(qi == 0), stop=False,
                                            skip_group_check=True)
                                    for qi in range(4):
                                        nc.tensor.matmul(
                                            tps_ps[:, qi, :],
                                            lhsT=att4[:, qi, :], rhs=ident,
                                            is_transpose=True,
                                            start=False, stop=(qi == 3),
                                            skip_group_check=True)
                                    tps = attp.tile(
                                        [128, 8, 128], bf16, tag="tpss")
                                    if "attcopyact" in opts:
                                        nc.scalar.copy(tps, tps_ps)
                                    else:
                                        nc.vector.tensor_copy(tps, tps_ps)
                                    attT4 = tps[:, 0:4, :]
                                    vp4 = tps[:, 4:8, :]
                                else:
                                    attT4_ps = ps2.tile(
                                        [128, 4, 128], bf16, tag="attT4")
                                    vp4_ps = ps2.tile(
                                        [128, 4, 128], bf16, tag="vp4")
                                    for qi in range(4):
                                        n0 = 32 * a + 8 * qi
                                        nc.tensor.matmul(
                                            attT4_ps[:, qi, :],
                                            lhsT=att4[:, qi, :], rhs=ident,
                                            is_transpose=True,
                                            start=(qi == 0), stop=(qi == 3),
                                            skip_group_check=True)
                                        v_v = qkv_slice("v", n0).rearrange(
                                            "d g t -> d (g t)")
                                        nc.tensor.matmul(
                                            vp4_ps[:, qi, :], lhsT=v_v,
                                            rhs=ident, is_transpose=True,
                                            start=(qi == 0), stop=(qi == 3),
                                            skip_group_check=True)
                                    attT4 = attp.tile(
                                        [128, 4, 128], bf16, tag="attT4s")
                                    if "attcopyact" in opts:
                                        nc.scalar.copy(attT4, attT4_ps)
                                    else:
                                        nc.vector.tensor_copy(attT4, attT4_ps)
                                    vp4 = attp.tile(
                                        [128, 4, 128], bf16, tag="vp4s")
                                    if "vpcopyact" in opts:
                                        nc.scalar.copy(vp4, vp4_ps)
                                    else:
                                        nc.vector.tensor_copy(vp4, vp4_ps)

                                o4_ps = ps2.tile(
                                    [128, 4, 128], f32, tag="o4",
                                    bufs=3 if "tpsmerge" in opts else None)
                                for qi in range(4):
                                    nc.tensor.matmul(
                                        o4_ps[:, qi, :],
                                        lhsT=vp4[:, qi, :],
                                        rhs=attT4[:, qi, :],
                                        start=(qi == 0), stop=(qi == 3),
                                        skip_group_check=True)
                                tc0 = 2 * a
                                half, tcl0 = tc0 // tw2, tc0 % tw2
                                dst = scr5[half][:, :, tcl0:tcl0 + 2, :].rearrange(
                                    "p (jb g) t h -> p t jb g h", jb=2)
                                nc.vector.tensor_copy(
                                    dst,
                                    o4_ps.rearrange(
                                        "p (tb jb) (g h) -> p tb jb g h",
                                        jb=2, h=16))
                            nquads_left = 0
                        else:
                            nquads_left = nquads
                        for g0 in range(
                                0 if "noatt" not in opts and nquads_left else 10**9,
                                nquads_left):
                            n0 = 8 * g0
                            j0, tc_ = n0 % 16, g0 // 2
                            q_v = qkv_sb["q"][:, n0:n0 + 8, :].rearrange(
                                "d g h -> d (g h)")
                            k_v = qkv_sb["k"][:, n0:n0 + 8, :].rearrange(
                                "d g h -> d (g h)")
                            qk_ps = ps2.tile([128, 128], f32, tag="qk")
                            att = attp.tile([128, 128], bf16, tag="att")
                            rs = attp.tile([128, 1], f32, tag="rs")
                            if "fastmask" in opts:
                                nc.tensor.matmul(
                                    qk_ps, lhsT=q_v, rhs=k_v,
                                    start=True, stop=False)
                                nc.tensor.matmul(
                                    qk_ps, lhsT=ml_sb, rhs=mr_sb,
                                    start=False, stop=True)
                                nc.scalar.activation(
                                    att, qk_ps, Exp, bias=ebias, accum_out=rs)
                            else:
                                nc.tensor.matmul(
                                    qk_ps, lhsT=q_v, rhs=k_v,
                                    start=True, stop=True)
                                if "noatt_dve" not in opts:
                                    nc.vector.tensor_add(qk_ps, qk_ps, mask_sb)
                                nc.scalar.activation(att, qk_ps, Exp, accum_out=rs)
                            if "noatt_dve" not in opts:
                                if "divnorm" in opts:
                                    nc.vector.tensor_scalar(
                                        att, att, rs, None,
                                        op0=mybir.AluOpType.divide)
                                else:
                                    rc = attp.tile([128, 1], f32, tag="rc")
                                    nc.vector.reciprocal(rc, rs)
                                    nc.vector.tensor_scalar_mul(att, att, rc)

                            attT_ps = ps2.tile([128, 128], bf16, tag="attT")
                            nc.tensor.transpose(attT_ps, att, ident)
                            attT = attp.tile([128, 128], bf16, tag="attTs")
                            nc.vector.tensor_copy(attT, attT_ps)

                            if vphoist:
                                vp = v2_sb[:, g0, :]
                            else:
                                v_v = qkv_sb["v"][:, n0:n0 + 8, :].rearrange(
                                    "d g t -> d (g t)")
                                vp_ps = ps2.tile([128, 128], bf16, tag="vp")
                                nc.tensor.transpose(vp_ps, v_v, ident)
                                vp = attp.tile([128, 128], bf16, tag="vps")
                                nc.vector.tensor_copy(vp, vp_ps)

                            o_ps = ps2.tile(
                                [128, 128], f32, tag="o",
                                bufs=2 if vphoist else None)
                            nc.tensor.matmul(
                                o_ps, lhsT=vp, rhs=attT, start=True, stop=True)
                            if "scrsplit" in opts:
                                tw2 = TW // 2
                                half, tcl = tc_ // tw2, tc_ % tw2
                                dst = scr5[half][:, j0:j0 + 8, tcl:tcl + 1, :]
                            else:
                                dst = scr4[:, j0:j0 + 8, :, tc_:tc_ + 1]
                            nc.vector.tensor_copy(
                                dst, o_ps.rearrange("p (g h) -> p g h", g=8))

            # ---------------- phase 3: output projection ----------------
            wo_v = w_d["o"].ap().rearrange("(jt jp) r -> jp jt r", jp=128)
            out_v = out_d.ap().rearrange("(rt rp) n -> rp rt n", rp=128)
            if "scrsplit" in opts:
                ch3 = npos // 2
                chunks = [(scrA, slice(0, ch3)), (scrB, slice(ch3, npos))]
            else:
                ch3 = CH
                chunks = [
                    (scr_sb, slice(c * CH, (c + 1) * CH)) for c in range(NCH)]
            if fuse:
                _c3 = tc.tile_pool(name="ps3", bufs=2, space="PSUM")
                ps3_h = _c3.__enter__()
                held.append(_c3)
                ps3_ctx = contextlib.nullcontext(ps3_h)
            else:
                ps3_ctx = tc.tile_pool(name="ps3", bufs=3, space="PSUM")
            with ps3_ctx as ps3:
                for rg in range(8):
                    w_sb = wpool.tile([128, 16, 256], bf16, tag="w")
                    nc.sync.dma_start(w_sb, wo_v[:, :, rg * 256:(rg + 1) * 256])
                    for ro in range(2):
                        r = rg * 2 + ro
                        for src, csl in chunks:
                            ps = ps3.tile([128, ch3], f32)
                            for j in range(16):
                                rhs = (src[:, j, :] if "scrsplit" in opts
                                       else src[:, j, csl])
                                nc.tensor.matmul(
                                    ps,
                                    lhsT=w_sb[:, j, ro * 128:(ro + 1) * 128],
                                    rhs=rhs,
                                    start=(j == 0), stop=(j == 15))
                            o_sb = outp.tile([128, ch3], f32)
                            nc.scalar.activation(
                                o_sb, ps, Ident, bias=b_sb["o"][:, r:r + 1])
                            nc.sync.dma_start(out_v[:, r, csl], o_sb)

            for cm in reversed(held):
                cm.__exit__(None, None, None)

    def body_v2(tc):
        """Dependency-restructured pipeline:

        q-proj, k-proj (m-outer, weights streamed once) -> attention part1
        (qk+mask matmul, exp, normalize -> att_all in SBUF; needs only q,k)
        emitted BEFORE v-proj so part1's ACT/DVE work overlaps v's dense PE
        matmuls -> part2 (transposes + att@v + scatter) -> o-projection in
        two passes (scrA, scrB; Wo streamed twice) so the list scheduler
        fills part2 latency bubbles with o-proj matmuls.
        """
        with (
            tc.tile_pool(name="consts", bufs=1) as consts,
            tc.tile_pool(name="wpool", bufs=2) as wpool,
            tc.tile_pool(name="vpool", bufs=1) as vpool,
            tc.tile_pool(name="attall", bufs=1) as attall,
            tc.tile_pool(name="attp", bufs=3) as attp,
            tc.tile_pool(name="tmp", bufs=2) as tmp,
            tc.tile_pool(name="outp", bufs=2) as outp,
        ):
            cos_sb = consts.tile([128, npos], f32)
            nc.sync.dma_start(cos_sb, cos_d.ap())
            sin_sb = consts.tile([128, npos], f32)
            nc.sync.dma_start(sin_sb, sin_d.ap())
            if "dvemask" in opts:
                mask4 = consts.tile([128, 4, 128], f32)
                for qi in range(4):
                    nc.sync.dma_start(mask4[:, qi, :], mask_d.ap())
            else:
                ml_sb = consts.tile([8, 128], bf16)
                nc.sync.dma_start(ml_sb, maskl_d.ap())
                mr4_sb = consts.tile([8, 512], bf16)
                nc.sync.dma_start(mr4_sb, maskr4_d.ap())
                ebias = consts.tile([128, 1], f32)
                nc.vector.memset(ebias, -MASKVAL)
            ident = consts.tile([128, 128], bf16)
            make_identity(nc, ident)
            b_sb = {}
            for p in ("q", "k", "v", "o"):
                b_sb[p] = consts.tile([128, 16], f32, tag=f"b_{p}", name=f"b_{p}")
                nc.sync.dma_start(b_sb[p], b_d[p].ap().rearrange("(t p) -> p t", p=128))

            att_all = attall.tile([128, NQ // 4, 4, 128], bf16)
            v_sb = vpool.tile([128, npos, 16], bf16, tag="qkv_v", name="qkv_v")

            with (
                tc.tile_pool(name="qkp", bufs=1) as qkp,
                tc.tile_pool(name="xp", bufs=1) as xp,
                tc.tile_pool(
                    name="ps1", bufs=4 if "wreuse" in opts else 3,
                    space="PSUM") as ps1,
                tc.tile_pool(name="psqk", bufs=3, space="PSUM") as psqk,
            ):
                xt_sb = xp.tile([128, 16, npos], bf16)
                xt_v = xt_d.ap().rearrange("(kt kp) n -> kp kt n", kp=128)
                nxc = max(NCH, 4)
                xcw = npos // nxc
                for xc in range(nxc):
                    csl = slice(xc * xcw, (xc + 1) * xcw)
                    nc.sync.dma_start(xt_sb[:, :, csl], xt_v[:, :, csl])

                qk_sb = {
                    p: qkp.tile([128, npos, 16], bf16,
                                tag=f"qkv_{p}", name=f"qkv_{p}")
                    for p in ("q", "k")
                }

                wreuse = "wreuse" in opts

                nobias = "nobias" in opts

                def evict(p, dst, ps, m, csl):
                    qdst = dst[:, csl, m]
                    if p == "v" or "norope" in opts:
                        if nobias:
                            nc.scalar.copy(qdst, ps)
                        else:
                            nc.scalar.activation(
                                qdst, ps, Ident, bias=b_sb[p][:, m:m + 1])
                    else:
                        if not nobias:
                            nc.vector.tensor_scalar_add(
                                ps, ps, b_sb[p][:, m:m + 1])
                        t1 = tmp.tile([128, CH], f32, tag="t1")
                        nc.vector.tensor_mul(t1, ps, cos_sb[:, csl])
                        tsw = tmp.tile([128, CH], f32, tag="tsw")
                        nc.scalar.copy(tsw[0:64, :], ps[64:128, :])
                        nc.scalar.copy(tsw[64:128, :], ps[0:64, :])
                        nc.vector.tensor_mul(tsw, tsw, sin_sb[:, csl])
                        nc.vector.tensor_add(qdst, t1, tsw)

                wlay = "wlay" in opts

                def proj(p, dst):
                    wv_d = w_d[p].ap().rearrange("(kt kp) e -> kp kt e", kp=128)
                    for mg in range(8):
                        w_sb = wpool.tile([128, 16, 256], bf16, tag="w")
                        if wlay:
                            nc.sync.dma_start(w_sb, wr_d[p].ap()[mg])
                        else:
                            nc.sync.dma_start(
                                w_sb, wv_d[:, :, mg * 256:(mg + 1) * 256])
                        for mo in range(2):
                            m = mg * 2 + mo
                            if wreuse:
                                # kt-outer: consecutive matmuls share the
                                # stationary operand (one weight load per
                                # kt feeds both column chunks)
                                pss = [
                                    ps1.tile([128, CH], f32, tag="ps1",
                                             name=f"ps1_{ch}")
                                    for ch in range(NCH)
                                ]
                                for kt in range(16):
                                    for ch in range(NCH):
                                        nc.tensor.matmul(
                                            pss[ch],
                                            lhsT=w_sb[:, kt, mo * 128:(mo + 1) * 128],
                                            rhs=xt_sb[:, kt, ch * CH:(ch + 1) * CH],
                                            start=(kt == 0), stop=(kt == 15))
                                for ch in range(NCH):
                                    evict(p, dst, pss[ch], m,
                                          slice(ch * CH, (ch + 1) * CH))
                            else:
                                for ch in range(NCH):
                                    csl = slice(ch * CH, (ch + 1) * CH)
                                    ps = ps1.tile([128, CH], f32, tag="ps1")
                                    for kt in range(16):
                                        nc.tensor.matmul(
                                            ps,
                                            lhsT=w_sb[:, kt, mo * 128:(mo + 1) * 128],
                                            rhs=xt_sb[:, kt, csl],
                                            start=(kt == 0), stop=(kt == 15))
                                    evict(p, dst, ps, m, csl)

                proj("q", qk_sb["q"])
                proj("k", qk_sb["k"])

                # attention part1: qk + mask -> exp -> normalize -> att_all
                for a in range(0 if "projonly" in opts else NQ // 4):
                    qk4 = psqk.tile([128, 4, 128], f32, tag="qk4")
                    for qi in range(4):
                        n0 = 32 * a + 8 * qi
                        q_v = qk_sb["q"][:, n0:n0 + 8, :].rearrange(
                            "d g h -> d (g h)")
                        k_v = qk_sb["k"][:, n0:n0 + 8, :].rearrange(
                            "d g h -> d (g h)")
                        nc.tensor.matmul(
                            qk4[:, qi, :], lhsT=q_v, rhs=k_v,
                            start=(qi == 0),
                            stop=("dvemask" in opts and qi == 3),
                            skip_group_check=True)
                    att4 = att_all[:, a, :, :]
                    if "dvemask" in opts:
                        nc.vector.tensor_add(qk4, qk4, mask4)
                        nc.scalar.activation(att4, qk4, Exp)
                    else:
                        nc.tensor.matmul(
                            qk4.rearrange("p q n -> p (q n)"),
                            lhsT=ml_sb, rhs=mr4_sb,
                            start=False, stop=True, skip_group_check=True)
                        nc.scalar.activation(att4, qk4, Exp, bias=ebias)
                    rs4 = attp.tile([128, 4], f32, tag="rs4")
                    nc.vector.reduce_sum(
                        out=rs4, in_=att4, axis=mybir.AxisListType.X)
                    rc4 = attp.tile([128, 4], f32, tag="rc4")
                    nc.vector.reciprocal(rc4, rs4)
                    for qi in range(4):
                        nc.vector.tensor_scalar_mul(
                            att4[:, qi, :], att4[:, qi, :], rc4[:, qi:qi + 1])

                proj("v", v_sb)

            # ---- part2 + o-projection, interleaved by the scheduler ----
            fgo = "fgo" in opts
            with (
                tc.tile_pool(name="scrp", bufs=2 if fgo else 1) as scrp,
                tc.tile_pool(name="wop", bufs=1) as wop,
                tc.tile_pool(name="ps2", bufs=2, space="PSUM") as ps2,
                tc.tile_pool(name="ps3", bufs=4, space="PSUM") as ps3,
            ):
                tw2 = TW // 2
                if fgo:
                    # rolling scr tiles of 256 cols; o-proj per chunk with
                    # Wo fully resident (loaded during part2's first groups)
                    scr_t = {}

                    def scr_tile(t):
                        if t not in scr_t:
                            s = scrp.tile([128, 16, 256], bf16, tag="scrt",
                                          name=f"scrt{t}")
                            scr_t[t] = s.rearrange(
                                "p j (t h) -> p j t h", h=16)
                        return scr_t[t]
                else:
                    scrA = scrp.tile([128, 16, npos // 2], bf16, tag="scrA")
                    scrB = scrp.tile([128, 16, npos // 2], bf16, tag="scrB")
                    scr5 = [
                        s.rearrange("p j (t h) -> p j t h", h=16)
                        for s in (scrA, scrB)
                    ]

                def part2(a):
                    tps_ps = ps2.tile([128, 8, 128], bf16, tag="tps")
                    for qi in range(4):
                        n0 = 32 * a + 8 * qi
                        v_v = v_sb[:, n0:n0 + 8, :].rearrange(
                            "d g t -> d (g t)")
                        nc.tensor.matmul(
                            tps_ps[:, 4 + qi, :], lhsT=v_v,
                            rhs=ident, is_transpose=True,
                            start=(qi == 0), stop=False,
                            skip_group_check=True)
                    for qi in range(4):
                        nc.tensor.matmul(
                            tps_ps[:, qi, :],
                            lhsT=att_all[:, a, qi, :], rhs=ident,
                            is_transpose=True,
                            start=False, stop=(qi == 3),
                            skip_group_check=True)
                    tps = attp.tile([128, 8, 128], bf16, tag="tpss")
                    nc.scalar.copy(tps, tps_ps)
                    o4_ps = ps2.tile([128, 4, 128], f32, tag="o4")
                    for qi in range(4):
                        nc.tensor.matmul(
                            o4_ps[:, qi, :],
                            lhsT=tps[:, 4 + qi, :],
                            rhs=tps[:, qi, :],
                            start=(qi == 0), stop=(qi == 3),
                            skip_group_check=True)
                    tc0 = 2 * a
                    if fgo:
                        tcl0 = tc0 % 16
                        dst = scr_tile(a // 8)[:, :, tcl0:tcl0 + 2, :].rearrange(
                            "p (jb g) t h -> p t jb g h", jb=2)
                    else:
                        half, tcl0 = tc0 // tw2, tc0 % tw2
                        dst = scr5[half][:, :, tcl0:tcl0 + 2, :].rearrange(
                            "p (jb g) t h -> p t jb g h", jb=2)
                    nc.vector.tensor_copy(
                        dst,
                        o4_ps.rearrange(
                            "p (tb jb) (g h) -> p tb jb g h",
                            jb=2, h=16))

                wo_v = w_d["o"].ap().rearrange("(jt jp) r -> jp jt r", jp=128)
                out_v = out_d.ap().rearrange("(rt rp) n -> rp rt n", rp=128)

                def oproj_pass(src, osl):
                    for rg in range(8):
                        w_sb = wpool.tile([128, 16, 256], bf16, tag="w")
                        if "wlay" in opts:
                            nc.sync.dma_start(w_sb, wr_d["o"].ap()[rg])
                        else:
                            nc.sync.dma_start(
                                w_sb, wo_v[:, :, rg * 256:(rg + 1) * 256])
                        for ro in range(2):
                            r = rg * 2 + ro
                            ps = ps3.tile([128, npos // 2], f32, tag="ps3")
                            for j in range(16):
                                nc.tensor.matmul(
                                    ps,
                                    lhsT=w_sb[:, j, ro * 128:(ro + 1) * 128],
                                    rhs=src[:, j, :],
                                    start=(j == 0), stop=(j == 15))
                            o_sb = outp.tile([128, npos // 2], out_dt)
                            if "nobias" in opts:
                                nc.scalar.copy(o_sb, ps)
                            else:
                                nc.scalar.activation(
                                    o_sb, ps, Ident, bias=b_sb["o"][:, r:r + 1])
                            nc.sync.dma_start(out_v[:, r, osl], o_sb)

                if "projonly" in opts:
                    nc.vector.memset(scrA, 0.0)
                    nc.vector.memset(scrB, 0.0)
                    oproj_pass(scrA, slice(0, npos // 2))
                    oproj_pass(scrB, slice(npos // 2, npos))
                elif fgo:
                    wo_sb = []
                    for rg in range(8):
                        wt = wop.tile([128, 16, 256], bf16,
                                      tag=f"wo{rg}", name=f"wo{rg}")
                        if "wlay" in opts:
                            nc.sync.dma_start(wt, wr_d["o"].ap()[rg])
                        else:
                            nc.sync.dma_start(
                                wt, wo_v[:, :, rg * 256:(rg + 1) * 256])
                        wo_sb.append(wt)
                    nchunk = NQ // 4 // 8          # part2 groups per chunk = 8
                    for t in range(nchunk):
                        for a in range(8 * t, 8 * t + 8):
                            part2(a)
                        src = scr_t[t]             # [128, 16, 16, 16] view
                        for rg in range(8):
                            for ro in range(2):
                                r = rg * 2 + ro
                                ps = ps3.tile([128, 256], f32, tag="ps3")
                                for j in range(16):
                                    nc.tensor.matmul(
                                        ps,
                                        lhsT=wo_sb[rg][:, j, ro * 128:(ro + 1) * 128],
                                        rhs=src[:, j, :, :].rearrange(
                                            "p t h -> p (t h)"),
                                        start=(j == 0), stop=(j == 15))
                                o_sb = outp.tile([128, 256], out_dt)
                                if "nobias" in opts:
                                    nc.scalar.copy(o_sb, ps)
                                else:
                                    nc.scalar.activation(
                                        o_sb, ps, Ident,
                                        bias=b_sb["o"][:, r:r + 1])
                                nc.sync.dma_start(
                                    out_v[:, r, t * 256:(t + 1) * 256], o_sb)
                else:
                    for a in range(NQ // 8):
                        part2(a)
                    oproj_pass(scrA, slice(0, npos // 2))
                    for a in range(NQ // 8, NQ // 4):
                        part2(a)
                    oproj_pass(scrB, slice(npos // 2, npos))

    with tile.TileContext(nc) as tc:
        for _ in range(reps):
            if "v2" in opts:
                body_v2(tc)
            else:
                body(tc)

    nc.compile()
    _NC_CACHE[key] = nc
    return nc


# ---------------------------------------------------------------------------
# Runner (PJRT via axon, cached jitted callable)
# ---------------------------------------------------------------------------

_RUNNER_CACHE = {}


def make_runner(nc, ncores=NCORES):
    """Returns run(in_maps) -> list of per-core output dicts.

    Mirrors bass2jax.run_bass_via_pjrt but caches the jitted callable and
    does NOT donate output buffers (kernel writes every output element), so
    repeated timed calls don't re-trace or re-transfer.
    """
    key = id(nc)
    if key in _RUNNER_CACHE:
        return _RUNNER_CACHE[key]

    import jax
    import numpy as _np
    from jax.sharding import Mesh, PartitionSpec
    from jax.experimental.shard_map import shard_map
    from concourse import mybir
    from concourse import bass2jax
    from concourse.bass2jax import (
        _bass_exec_p, install_neuronx_cc_hook, partition_id_tensor)

    install_neuronx_cc_hook()

    partition_name = (
        nc.partition_id_tensor.name if nc.partition_id_tensor else None)
    in_names, out_names, out_avals, zero_outs = [], [], [], []
    for alloc in nc.m.functions[0].allocations:
        if not isinstance(alloc, mybir.MemoryLocationSet):
            continue
        name = alloc.memorylocations[0].name
        if alloc.kind == "ExternalInput":
            if name == partition_name:
                continue
            in_names.append(name)
        elif alloc.kind == "ExternalOutput":
            shape = tuple(alloc.tensor_shape)
            dtype = mybir.dt.np(alloc.dtype)
            out_names.append(name)
            out_avals.append(jax.core.ShapedArray(shape, dtype))
            zero_outs.append(_np.zeros(shape, dtype))
    n_params = len(in_names)
    all_in_names = in_names + out_names
    if partition_name is not None:
        all_in_names = all_in_names + [partition_name]

    def _body(*args):
        operands = list(args)
        if partition_name is not None:
            operands.append(partition_id_tensor())
        outs = _bass_exec_p.bind(
            *operands,
            out_avals=tuple(out_avals),
            in_names=tuple(all_in_names),
            out_names=tuple(out_names),
            lowering_input_output_aliases=(),
            sim_require_finite=True,
            sim_require_nnan=True,
            nc=nc,
        )
        return tuple(outs)

    devices = jax.devices()[:ncores]
    mesh = Mesh(np.asarray(devices), ("core",))
    n_outs = len(out_names)
    jitted = jax.jit(
        shard_map(
            _body, mesh=mesh,
            in_specs=(PartitionSpec("core"),) * (n_params + n_outs),
            out_specs=(PartitionSpec("core"),) * n_outs,
            check_rep=False,
        ),
        keep_unused=True,
    )

    zeros_dev = [
        jax.device_put(
            _np.zeros((ncores * z.shape[0], *z.shape[1:]), z.dtype))
        for z in zero_outs
    ]

    def put(in_maps):
        concat = [
            _np.concatenate([_np.asarray(m[name]) for m in in_maps], axis=0)
            for name in in_names
        ]
        return [jax.device_put(a) for a in concat]

    def run_dev(in_dev):
        outs = jitted(*in_dev, *zeros_dev)
        jax.block_until_ready(outs)
        return outs

    def run(in_maps):
        outs = run_dev(put(in_maps))
        res = []
        for c in range(len(in_maps)):
            res.append({
                name: _np.asarray(outs[i]).reshape(
                    len(in_maps), *out_avals[i].shape)[c]
                for i, name in enumerate(out_names)
            })
        return res

    run.put = put
    run.run_dev = run_dev
    run.out_names = out_names
    _RUNNER_CACHE[key] = run
    return run


DEFAULT_OPTS = frozenset({"v2"})


def kernel(**inputs) -> np.ndarray:
    inputs = {k: np.asarray(v) for k, v in inputs.items()}
    opts = DEFAULT_OPTS
    in_maps, meta = prepare_host(**inputs)
    nc = build_nc(NPOS, opts=opts)
    run = make_runner(nc, NCORES)
    outs = run(in_maps)
    layout = "t_h" if ("scrsplit" in opts or "v2" in opts) else "h_t"
    return assemble_output(outs, meta, NPOS, layout)

